# revision 2
# baseline (speedup 1.0000x reference)
"""Trainium2 Bass kernel for nn_ClusterAttn (vq_codebook).

Strategy (8 NeuronCores, SPMD):
  - Shard the h axis (128) into 8 slabs of 16 rows; windows (4^3) stay core-local.
  - Host-side prep: per-core channel-major bf16 slab of x (with h halo) for the
    dwc conv; natural fp32 slab for the residual; pre-transposed weights.
  - Phase A (dwc conv): per (d, h-row) matmul  Z[w, o] = sum_c x[c,*] * w27[c, o]
    (o = 27 taps), then 27 free-dim-shifted DVE adds produce dnx in a
    window-friendly layout. w+-1 shifts are pre-materialized by two DMA copies.
  - Phase B (cluster attention): all matmuls on 64-wide fea tiles; softmax over
    windows needs two tiny AllReduces (sum of squares; exp-sums + soft counts).
    No max-subtraction (logits are tiny; validated numerically).
  - Phase C (upc conv + residual): out volume -> DRAM bounce -> ov tile (d on
    partitions) -> 27-row im2col built by DMA -> one matmul per (d,h) row,
    + fp32 residual, streamed back to HBM.  h-halo of ov comes from an
    AllGather of window-face slices.
"""

import os
import sys
from contextlib import ExitStack
from dataclasses import dataclass

import numpy as np

for _p in ("/opt/trn_rl_repo",):
    if os.path.isdir(_p) and _p not in sys.path:
        sys.path.insert(0, _p)

os.environ.setdefault("MYCRO_LOCAL_CACHE", "1")

import ml_dtypes  # noqa: E402
import concourse.bass as bass  # noqa: E402
import concourse.tile as tile  # noqa: E402
from concourse import bacc, mybir  # noqa: E402
from concourse import bass_utils  # noqa: E402
from concourse.masks import make_identity  # noqa: E402

F32 = mybir.dt.float32
BF16 = mybir.dt.bfloat16
AF = mybir.ActivationFunctionType
ALU = mybir.AluOpType

C = 96          # channels
P = 4           # window edge
FEAD = 64       # P^3
K = 64          # clusters
NCORES = 8
W = 128         # w extent == partition count


@dataclass(frozen=True)
class Cfg:
    D: int = 32      # d extent
    HL: int = 16     # h rows per core (total H = 8*HL)

    @property
    def HZ(self):     # h rows incl halo
        return self.HL + 2

    @property
    def DZ(self):     # d extent incl halo
        return self.D + 2

    @property
    def WD(self):
        return self.D // P

    @property
    def WHL(self):
        return self.HL // P

    @property
    def WWN(self):
        return W // P

    @property
    def NLOC(self):  # windows per core
        return self.WD * self.WHL * self.WWN

    @property
    def NT(self):    # 128-row tiles of local windows
        return self.NLOC // 128


FULL = Cfg()

_BUILD_CACHE: dict = {}


def mkap(base, extra_off, dims):
    """Manual AP on the same tensor; offsets/steps in flat element units
    (partition pitch == free size)."""
    return bass.AP(tensor=base.tensor, offset=base.offset + extra_off, ap=dims)


def build_module(cfg: Cfg):
    D, HL, HZ, DZ = cfg.D, cfg.HL, cfg.HZ, cfg.DZ
    WD, WHL, WWN, NLOC, NT = cfg.WD, cfg.WHL, cfg.WWN, cfg.NLOC, cfg.NT
    WZP = W + 2
    PHZ = DZ * HZ           # per-o plane size in Z storage

    nc = bacc.Bacc("TRN2", target_bir_lowering=False, debug=False,
                   num_devices=NCORES)

    # ---------------- I/O ----------------
    xt = nc.dram_tensor("xt", [C, D, HZ, W], BF16, kind="ExternalInput").ap()
    xr = nc.dram_tensor("xr", [D, HL, W, C], F32, kind="ExternalInput").ap()
    w27 = nc.dram_tensor("w27", [C, 27], BF16, kind="ExternalInput").ap()
    dwcb = nc.dram_tensor("dwcb", [1, 1], F32, kind="ExternalInput").ap()
    centt = nc.dram_tensor("centt", [FEAD, K], F32, kind="ExternalInput").ap()
    qwt = nc.dram_tensor("qwt", [FEAD, FEAD], F32, kind="ExternalInput").ap()
    qb = nc.dram_tensor("qb", [FEAD, 1], F32, kind="ExternalInput").ap()
    kvk = nc.dram_tensor("kvk", [FEAD + 1, FEAD], F32, kind="ExternalInput").ap()
    kvv = nc.dram_tensor("kvv", [FEAD + 1, FEAD], F32, kind="ExternalInput").ap()
    upw = nc.dram_tensor("upw", [27, C], BF16, kind="ExternalInput").ap()
    y = nc.dram_tensor("y", [D, HL, W, C], F32, kind="ExternalOutput").ap()

    with tile.TileContext(nc) as tc, ExitStack() as ctx:
        _body(ctx, tc, cfg, xt, xr, w27, dwcb, centt, qwt, qb, kvk, kvv, upw, y)
    nc.compile()
    return nc


def _body(ctx, tc, cfg, xt, xr, w27, dwcb, centt, qwt, qb, kvk, kvv, upw, y):
    nc = tc.nc
    D, HL, HZ, DZ = cfg.D, cfg.HL, cfg.HZ, cfg.DZ
    WD, WHL, WWN, NLOC, NT = cfg.WD, cfg.WHL, cfg.WWN, cfg.NLOC, cfg.NT
    WZP = W + 2
    PHZ = DZ * HZ
    EDGE = WWN * WD * 16          # per-edge halo elements (ww, wd, pd, pw)

    consts = ctx.enter_context(tc.tile_pool(name="consts", bufs=1))
    small = ctx.enter_context(tc.tile_pool(name="small", bufs=1))
    dram = ctx.enter_context(tc.tile_pool(name="dram", bufs=1, space="DRAM"))
    psA = ctx.enter_context(tc.tile_pool(name="psA", bufs=4, space="PSUM"))

    # ---------------- constants ----------------
    w27s = consts.tile([C, 27], BF16)
    nc.sync.dma_start(w27s[:], w27[:])
    upws = consts.tile([27, C], BF16)
    nc.sync.dma_start(upws[:], upw[:])
    centts = consts.tile([FEAD, K], F32)
    nc.sync.dma_start(centts[:], centt[:])
    qwts = consts.tile([FEAD, FEAD], F32)
    nc.sync.dma_start(qwts[:], qwt[:])
    qbs = consts.tile([FEAD, 1], F32)
    nc.sync.dma_start(qbs[:], qb[:])
    kvks = consts.tile([FEAD + 1, FEAD], F32)
    nc.sync.dma_start(kvks[:], kvk[:])
    kvvs = consts.tile([FEAD + 1, FEAD], F32)
    nc.sync.dma_start(kvvs[:], kvv[:])
    dwcb1 = consts.tile([1, 1], F32)
    nc.sync.dma_start(dwcb1[:], dwcb[:])
    dwcbb = consts.tile([W, 1], F32)
    nc.gpsimd.partition_broadcast(dwcbb[:], dwcb1[:])
    ident = consts.tile([FEAD, FEAD], F32)
    make_identity(nc, ident[:])

    # persistent mid-size tensors
    dnx = small.tile([W, P, P, WD, WHL], F32)        # (w; pd, ph, wd, wh)
    feat = small.tile([FEAD, NLOC], F32)             # fea^T (j, n) n=(ww,wd,wh)
    fa = small.tile([128, NT, FEAD + 1], F32)        # fea (n, j | 1)
    ee = small.tile([128, NT, K], F32)               # exp(logits) (n, k)
    sq = small.tile([FEAD, NLOC], F32)               # scratch for Square
    ov = small.tile([DZ, HZ, WZP], BF16)             # out volume (d; h, w)

    # DRAM bounce buffers
    ar1_in = dram.tile([FEAD, 1], F32)
    ar1_out = dram.tile([FEAD, 1], F32, addr_space="Shared")
    ar2_in = dram.tile([K, FEAD + 1], F32)
    ar2_out = dram.tile([K, FEAD + 1], F32, addr_space="Shared")
    ovd = dram.tile([NLOC, FEAD], F32)
    ovd2 = dram.tile([WHL, WD, P, P, W], F32)   # (wh, wd, pd, ph, w) true-w order
    ag_in = dram.tile([2, EDGE], F32)
    ag_out = dram.tile([NCORES, 2, EDGE], F32, addr_space="Shared")

    # ================= PHASE A: dwc conv =================
    with tc.tile_pool(name="xin", bufs=3) as xpool, \
         tc.tile_pool(name="zps", bufs=4, space="PSUM") as zps, \
         tc.tile_pool(name="zsb", bufs=1) as zpool:

        zsb = zpool.tile([W, 9, 3, DZ, HZ], F32)     # Z (w; g, ow, dz, hz)
        zp = zpool.tile([W, 9, DZ, HZ], F32)         # Z shifted w+1 (ow=+1 taps)
        zm = zpool.tile([W, 9, DZ, HZ], F32)         # Z shifted w-1 (ow=-1 taps)

        # boundary zeros: d-halo planes of Z; full zero-init of the shifted
        # copies (covers their w-edge and d-halo rows in one go)
        nc.vector.memset(zsb[:, :, :, 0, :], 0.0)
        nc.vector.memset(zsb[:, :, :, DZ - 1, :], 0.0)
        nc.vector.memset(zp[:], 0.0)
        nc.vector.memset(zm[:], 0.0)

        for d in range(D):
            xin = xpool.tile([C, HZ, W], BF16)
            nc.sync.dma_start(xin[:], xt[:, d, :, :])
            for hb0 in range(0, HZ, 4):
                nr = min(4, HZ - hb0)
                ps = zps.tile([W, 4 * 27], F32, tag="zps")
                for i in range(nr):
                    nc.tensor.matmul(ps[:, i * 27:(i + 1) * 27],
                                     lhsT=xin[:, hb0 + i, :], rhs=w27s[:],
                                     start=True, stop=True)
                # copy psum -> zsb (o-major planes), strided dest
                src = ps[:, 0:nr * 27].rearrange("p (h g w3) -> p h g w3",
                                                 g=9, w3=3)
                dst = mkap(zsb[:], (d + 1) * HZ + hb0,
                           [[9 * 3 * PHZ, W], [1, nr], [3 * PHZ, 9], [PHZ, 3]])
                nc.scalar.copy(dst, src)

        # w-shifted copies.  Partitions are in permuted order w' = pw*32+ww
        # (true w = 4*ww+pw), so a +-1 shift in true w becomes two
        # consecutive-partition-range copies (+-32, and the pw wraparound).
        WQ = W // P   # 32
        for half in range(2):
            dz0 = 1 + half * (D // 2)
            ndz = D // 2
            span = [1, ndz * HZ]

            def shcopy(dstt, dst_fsz, dst_p0, nparts, src_p0, owi):
                nc.sync.dma_start(
                    out=mkap(dstt, dst_p0 * 9 * PHZ + dz0 * HZ,
                             [[9 * PHZ, nparts], [PHZ, 9], list(span)]),
                    in_=mkap(zsb[:], src_p0 * 27 * PHZ + owi * PHZ + dz0 * HZ,
                             [[27 * PHZ, nparts], [3 * PHZ, 9], list(span)]))

            # zp[p] = Z[w(p)+1]: dest [0,3*WQ) <- src [WQ,4*WQ);
            #                    dest [3*WQ, 4*WQ-1) <- src [1, WQ)
            shcopy(zp[:], 9 * PHZ, 0, 3 * WQ, WQ, 2)
            shcopy(zp[:], 9 * PHZ, 3 * WQ, WQ - 1, 1, 2)
            # zm[p] = Z[w(p)-1]: dest [WQ,4*WQ) <- src [0,3*WQ);
            #                    dest [1, WQ) <- src [3*WQ, 4*WQ-1)
            shcopy(zm[:], 9 * PHZ, WQ, 3 * WQ, 0, 0)
            shcopy(zm[:], 9 * PHZ, 1, WQ - 1, 3 * WQ, 0)

        # shifted-sum -> dnx, split per pd so every DVE op has <=3 free dims
        def zterm(g, ow, pd):
            od, oh = g // 3 - 1, g % 3 - 1
            if ow == 1:
                base, plane = zsb[:], (g * 3 + 1) * PHZ
            elif ow == 2:
                base, plane = zp[:], g * PHZ
            else:
                base, plane = zm[:], g * PHZ
            off = plane + (pd + od + 1) * HZ + (oh + 1)
            return mkap(base, off,
                        [[base.ap[0][0], W], [1, P], [4 * HZ, WD], [P, WHL]])

        for pd in range(P):
            acc = dnx[:, pd, :, :, :]
            nc.vector.tensor_add(acc, zterm(0, 0, pd), zterm(0, 1, pd))
            for g in range(9):
                for ow in range(3):
                    if g == 0 and ow in (0, 1):
                        continue
                    nc.vector.tensor_add(acc, acc, zterm(g, ow, pd))
        # + dwc bias (flat 2-D view; per-partition scalar broadcast)
        nc.vector.tensor_scalar_add(
            dnx[:].rearrange("p a b c d -> p (a b c d)"),
            dnx[:].rearrange("p a b c d -> p (a b c d)"), dwcbb[:])

    # ================= window partition =================
    # feat[j, n]; n = ww*(WD*WHL) + wd*WHL + wh ; j = pd*16 + ph*4 + pw
    NWIN_D = WD * WHL
    fsz = P * P * NWIN_D                # free size of dnx
    for pd in range(P):
        for ph in range(P):
            for pw in range(P):
                # dnx partition w' = pw*WQ2+ww ; free = (pd, ph, wd, wh)
                WQ2 = W // P
                j = pd * 16 + ph * 4 + pw
                off = (pw * WQ2) * fsz + pd * (P * NWIN_D) + ph * NWIN_D
                src = mkap(dnx[:], off, [[fsz, WWN], [1, NWIN_D]])
                nc.sync.dma_start(out=feat[j:j + 1, :], in_=src)

    # ---- sumsq -> AllReduce -> rnorm ----
    su = small.tile([FEAD, 1], F32)
    nc.scalar.activation(sq[:], feat[:], AF.Square, accum_out=su[:])
    nc.sync.dma_start(ar1_in[:], su[:])
    nc.gpsimd.collective_compute("AllReduce", ALU.add,
                                 replica_groups=[list(range(NCORES))],
                                 ins=[ar1_in.opt()], outs=[ar1_out.opt()])
    rn = small.tile([FEAD, 1], F32)
    nc.sync.dma_start(rn[:], ar1_out[:])
    nc.scalar.sqrt(rn[:], rn[:])
    nc.vector.tensor_scalar_max(rn[:], rn[:], 1e-12)
    nc.vector.reciprocal(rn[:], rn[:])

    # fea (n-major) tiles + ones column, via PE transpose
    nc.vector.memset(fa[:, :, FEAD:FEAD + 1], 1.0)
    for t in range(NT):
        pt = psA.tile([128, FEAD], F32, tag="pb")
        nc.tensor.transpose(pt[:], feat[:, t * 128:(t + 1) * 128], ident[:])
        nc.scalar.copy(fa[:, t, 0:FEAD], pt[:])

    # ---- logits, exp, A matrix ----
    cst = small.tile([FEAD, K], F32)
    nc.vector.tensor_scalar_mul(cst[:], centts[:], rn[:])
    for t in range(NT):
        lg = psA.tile([128, K], F32, tag="pb")
        nc.tensor.matmul(lg[:], lhsT=feat[:, t * 128:(t + 1) * 128],
                         rhs=cst[:], start=True, stop=True)
        nc.scalar.activation(ee[:, t, :], lg[:], AF.Exp)

    aps = psA.tile([K, FEAD + 1], F32, tag="pb")
    for t in range(NT):
        nc.tensor.matmul(aps[:], lhsT=ee[:, t, :], rhs=fa[:, t, :],
                         start=(t == 0), stop=(t == NT - 1))
    asb = small.tile([K, FEAD + 1], F32)
    nc.scalar.copy(asb[:], aps[:])
    nc.sync.dma_start(ar2_in[:], asb[:])
    nc.gpsimd.collective_compute("AllReduce", ALU.add,
                                 replica_groups=[list(range(NCORES))],
                                 ins=[ar2_in.opt()], outs=[ar2_out.opt()])
    ag = small.tile([K, FEAD + 1], F32)
    nc.sync.dma_start(ag[:], ar2_out[:])

    # ---- new centroids, k/v, q ----
    rs = small.tile([K, 1], F32)
    nc.vector.reciprocal(rs[:], ag[:, FEAD:FEAD + 1])
    nc.vector.tensor_scalar_mul(ag[:, 0:FEAD], ag[:, 0:FEAD], rs[:])
    nct = small.tile([FEAD + 1, K], F32)
    ncp = psA.tile([FEAD, K], F32, tag="pb")
    nc.tensor.transpose(ncp[:], ag[:, 0:FEAD], ident[:])
    nc.vector.tensor_scalar_mul(nct[0:FEAD, :], ncp[:], rn[:])
    nc.vector.memset(nct[FEAD:FEAD + 1, :], 1.0)

    kt = small.tile([FEAD, K], F32)
    kp = psA.tile([FEAD, K], F32, tag="pb")
    nc.tensor.matmul(kp[:], lhsT=kvks[:], rhs=nct[:], start=True, stop=True)
    nc.scalar.copy(kt[:], kp[:])

    va = small.tile([K, FEAD + 1], F32)
    vp = psA.tile([K, FEAD], F32, tag="pb")
    nc.tensor.matmul(vp[:], lhsT=nct[:], rhs=kvvs[:], start=True, stop=True)
    nc.scalar.copy(va[:, 0:FEAD], vp[:])
    nc.vector.memset(va[:, FEAD:FEAD + 1], 1.0)

    qws = small.tile([FEAD, FEAD], F32)
    nc.vector.tensor_scalar_mul(qws[:], qwts[:], rn[:])
    qt = small.tile([FEAD, NLOC], F32)
    e2 = small.tile([K, NLOC], F32)
    CH = 512 if NLOC % 512 == 0 else 128
    for h0 in range(0, NLOC, CH):
        qp = psA.tile([FEAD, CH], F32, tag="pb")
        nc.tensor.matmul(qp[:], lhsT=qws[:], rhs=feat[:, h0:h0 + CH],
                         start=True, stop=True)
        nc.scalar.activation(qt[:, h0:h0 + CH], qp[:], AF.Identity, bias=qbs[:])
    for h0 in range(0, NLOC, CH):
        qk = psA.tile([K, CH], F32, tag="pb")
        nc.tensor.matmul(qk[:], lhsT=kt[:], rhs=qt[:, h0:h0 + CH],
                         start=True, stop=True)
        nc.scalar.activation(e2[:, h0:h0 + CH], qk[:], AF.Exp,
                             scale=float(FEAD) ** -0.5)

    # ---- attention output tiles -> DRAM dump + halo faces ----
    for t in range(NT):
        op = psA.tile([128, FEAD + 1], F32, tag="pb")
        nc.tensor.matmul(op[:], lhsT=e2[:, t * 128:(t + 1) * 128], rhs=va[:],
                         start=True, stop=True)
        rc = small.tile([128, 1], F32, tag="rc")
        nc.vector.reciprocal(rc[:], op[:, FEAD:FEAD + 1])
        ot = small.tile([128, FEAD], F32, tag="ot")
        nc.vector.tensor_scalar_mul(ot[:], op[:, 0:FEAD], rc[:])
        nc.sync.dma_start(ovd[t * 128:(t + 1) * 128, :], ot[:])

    # reorder dump (n, j) -> volume-row layout (wh, wd, pd, ph, w) in DRAM
    for wh in range(WHL):
        for pd in range(P):
            for ph in range(P):
                nc.sync.dma_start(
                    out=mkap(ovd2[:],
                             ((wh * WD) * 16 + pd * P + ph) * W,
                             [[16 * W, WD], [P, WWN], [1, P]]),
                    in_=mkap(ovd[:],
                             wh * FEAD + pd * 16 + ph * P,
                             [[WHL * FEAD, WD], [NWIN_D * FEAD, WWN], [1, P]]))
    # halo faces (one contiguous-w call per edge); layout [edge][wd][pd][w]
    for edge in range(2):
        wh_e = 0 if edge == 0 else WHL - 1
        ph_e = 0 if edge == 0 else P - 1
        nc.sync.dma_start(
            out=mkap(ag_in[:], edge * EDGE, [[2 * EDGE, 1], [1, EDGE]]),
            in_=mkap(ovd2[:], ((wh_e * WD) * 16 + ph_e) * W,
                     [[16 * W, WD], [P * W, P], [1, W]]))

    nc.gpsimd.collective_compute("AllGather", ALU.bypass,
                                 replica_groups=[list(range(NCORES))],
                                 ins=[ag_in.opt()], outs=[ag_out.opt()])

    # ---- build ov (bf16) ----
    nc.vector.memset(ov[:], 0.0)
    OVF = HZ * WZP
    for pd in range(P):
        for ph in range(P):
            for wd in range(WD):
                dz = 4 * wd + pd + 1
                dst = mkap(ov[:], dz * OVF + (ph + 1) * WZP + 1,
                           [[OVF, 1], [P * WZP, WHL], [1, W]])
                src = mkap(ovd2[:], (wd * 16 + pd * P + ph) * W,
                           [[WD * 16 * W, WHL], [1, W]])
                nc.gpsimd.dma_start(out=dst, in_=src)
    # halo rows from AllGather (dynamic rank offsets, edge cores skip)
    pid = nc.partition_id()
    for pd in range(P):
        for wd in range(WD):
            dz = 4 * wd + pd + 1
            # low halo row hz=0 <- core pid-1 high edge
            src_off = (pid - 1) * (2 * EDGE) + 1 * EDGE + (wd * P + pd) * W
            dst = mkap(ov[:], dz * OVF + 0 * WZP + 1, [[OVF, 1], [1, W]])
            src = mkap(ag_out[:], src_off, [[W, 1], [1, W]])
            nc.gpsimd.dma_start(out=dst, in_=src, cond=(pid >= 1))
            # high halo row hz=HZ-1 <- core pid+1 low edge
            src_off2 = (pid + 1) * (2 * EDGE) + 0 * EDGE + (wd * P + pd) * W
            dst2 = mkap(ov[:], dz * OVF + (HZ - 1) * WZP + 1,
                        [[OVF, 1], [1, W]])
            src2 = mkap(ag_out[:], src_off2, [[W, 1], [1, W]])
            nc.gpsimd.dma_start(out=dst2, in_=src2, cond=(pid <= NCORES - 2))

    # ================= PHASE C: upc conv + residual =================
    DB = 4  # d rows per im2col block
    with tc.tile_pool(name="i2c", bufs=2) as cpool, \
         tc.tile_pool(name="xrp", bufs=4) as xrp, \
         tc.tile_pool(name="yp", bufs=4) as yp, \
         tc.tile_pool(name="psC", bufs=4, space="PSUM") as psC:
        for db in range(D // DB):
            i2c = cpool.tile([27, DB * HL * W], BF16)
            for o in range(27):
                od, oh, ow = o // 9 - 1, (o // 3) % 3 - 1, o % 3 - 1
                src = mkap(ov[:], (db * DB + od + 1) * OVF
                           + (oh + 1) * WZP + (ow + 1),
                           [[OVF, DB], [WZP, HL], [1, W]])
                nc.sync.dma_start(out=i2c[o:o + 1, :], in_=src)
            for dd in range(DB):
                d = db * DB + dd
                for hq in range(HL // 4):
                    psc = psC.tile([W, 4 * C], F32, tag="psc")
                    for i in range(4):
                        h = hq * 4 + i
                        nc.tensor.matmul(
                            psc[:, i * C:(i + 1) * C],
                            lhsT=i2c[:, (dd * HL + h) * W:(dd * HL + h + 1) * W],
                            rhs=upws[:], start=True, stop=True)
                    xr4 = xrp.tile([W, 4, C], F32)
                    nc.sync.dma_start(
                        out=xr4[:],
                        in_=mkap(xr[:], (d * HL + hq * 4) * W * C,
                                 [[C, W], [W * C, 4], [1, C]]))
                    ysb = yp.tile([W, 4 * C], F32)
                    nc.vector.tensor_add(ysb[:], psc[:],
                                         xr4[:].rearrange("p a b -> p (a b)"))
                    nc.sync.dma_start(
                        out=mkap(y[:], (d * HL + hq * 4) * W * C,
                                 [[C, W], [W * C, 4], [1, C]]),
                        in_=ysb[:])


# ======================= host side =======================

def _prep_inputs(cfg: Cfg, inputs):
    x = np.asarray(inputs["x"], np.float32)[0]          # (D, H, W, C)
    D, H = cfg.D, NCORES * cfg.HL
    assert x.shape == (D, H, W, C), x.shape
    HLp = cfg.HL

    xpad = np.zeros((D, H + 2, W, C), np.float32)
    xpad[:, 1:H + 1] = x
    dwc_w = np.asarray(inputs["dwc_w"], np.float32)
    upc_w = np.asarray(inputs["upc_w"], np.float32)
    q_w = np.asarray(inputs["q_w"], np.float32)
    kv_w = np.asarray(inputs["kv_w"], np.float32)
    q_b = np.asarray(inputs["q_b"], np.float32)
    kv_b = np.asarray(inputs["kv_b"], np.float32)
    upc_b = np.asarray(inputs["upc_b"], np.float32)
    cent = np.asarray(inputs["centroids"], np.float32)

    w27 = np.ascontiguousarray(dwc_w[0].reshape(C, 27)).astype(ml_dtypes.bfloat16)
    upwt = np.ascontiguousarray(upc_w[:, 0].reshape(C, 27).T).astype(ml_dtypes.bfloat16)
    dwcb = np.asarray(inputs["dwc_b"], np.float32).reshape(1, 1)
    centT = np.ascontiguousarray(cent.T)
    qwT = np.ascontiguousarray(q_w.T)
    qbv = q_b.reshape(FEAD, 1)
    kvk = np.concatenate([kv_w[0:FEAD].T, kv_b[None, 0:FEAD]], 0)
    kvv = np.concatenate([kv_w[FEAD:2 * FEAD].T, kv_b[None, FEAD:2 * FEAD]], 0)
    kvk = np.ascontiguousarray(kvk)
    kvv = np.ascontiguousarray(kvv)

    in_maps = []
    for k in range(NCORES):
        h0 = k * HLp
        slab = xpad[:, h0:h0 + HLp + 2]                  # (D, HZ, W, C)
        worder = np.array([4 * (p % 32) + p // 32 for p in range(W)])
        slab = slab[:, :, worder, :]
        xt = np.ascontiguousarray(slab.transpose(3, 0, 1, 2)).astype(
            ml_dtypes.bfloat16)
        xrs = np.ascontiguousarray(x[:, h0:h0 + HLp]) + upc_b[None, None, None, :]
        in_maps.append({
            "xt": xt, "xr": xrs.astype(np.float32), "w27": w27, "dwcb": dwcb,
            "centt": centT, "qwt": qwT, "qb": qbv, "kvk": kvk, "kvv": kvv,
            "upw": upwt,
        })
    return in_maps


def _get_built(cfg: Cfg):
    if cfg not in _BUILD_CACHE:
        _BUILD_CACHE[cfg] = build_module(cfg)
    return _BUILD_CACHE[cfg]


def _postprocess(cfg: Cfg, res):
    ys = [res.results[k]["y"] for k in range(NCORES)]
    yfull = np.concatenate(ys, axis=1)                   # (D, H, W, C)
    return yfull[None].astype(np.float32)


def kernel(**inputs):
    cfg = FULL
    nc = _get_built(cfg)
    in_maps = _prep_inputs(cfg, inputs)
    res = bass_utils.run_bass_kernel_spmd(nc, in_maps,
                                          core_ids=list(range(NCORES)))
    return _postprocess(cfg, res)



# revision 14
# speedup vs baseline: 1.2061x; 1.2061x over previous
"""Trainium2 Bass kernel for nn_ClusterAttn (vq_codebook).

Strategy (8 NeuronCores, SPMD):
  - Shard the h axis (128) into 8 slabs of 16 rows; windows (4^3) stay core-local.
  - Host-side prep: per-core channel-major bf16 slab of x (with h halo) for the
    dwc conv; natural fp32 slab for the residual; pre-transposed weights.
  - Phase A (dwc conv): per (d, h-row) matmul  Z[w, o] = sum_c x[c,*] * w27[c, o]
    (o = 27 taps), then 27 free-dim-shifted DVE adds produce dnx in a
    window-friendly layout. w+-1 shifts are pre-materialized by two DMA copies.
  - Phase B (cluster attention): all matmuls on 64-wide fea tiles; softmax over
    windows needs two tiny AllReduces (sum of squares; exp-sums + soft counts).
    No max-subtraction (logits are tiny; validated numerically).
  - Phase C (upc conv + residual): out volume -> DRAM bounce -> ov tile (d on
    partitions) -> 27-row im2col built by DMA -> one matmul per (d,h) row,
    + fp32 residual, streamed back to HBM.  h-halo of ov comes from an
    AllGather of window-face slices.
"""

import os
import sys
from contextlib import ExitStack
from dataclasses import dataclass

import numpy as np

for _p in ("/opt/trn_rl_repo",):
    if os.path.isdir(_p) and _p not in sys.path:
        sys.path.insert(0, _p)

os.environ.setdefault("MYCRO_LOCAL_CACHE", "1")

import ml_dtypes  # noqa: E402
import concourse.bass as bass  # noqa: E402
import concourse.tile as tile  # noqa: E402
from concourse import bacc, mybir  # noqa: E402
from concourse import bass_utils  # noqa: E402
from concourse.masks import make_identity  # noqa: E402

F32 = mybir.dt.float32
BF16 = mybir.dt.bfloat16
AF = mybir.ActivationFunctionType
ALU = mybir.AluOpType

C = 96          # channels
P = 4           # window edge
FEAD = 64       # P^3
K = 64          # clusters
NCORES = 8
W = 128         # w extent == partition count


@dataclass(frozen=True)
class Cfg:
    D: int = 32      # d extent
    HL: int = 16     # h rows per core (total H = 8*HL)

    @property
    def HZ(self):     # h rows incl halo
        return self.HL + 2

    @property
    def DZ(self):     # d extent incl halo
        return self.D + 2

    @property
    def WD(self):
        return self.D // P

    @property
    def WHL(self):
        return self.HL // P

    @property
    def WWN(self):
        return W // P

    @property
    def NLOC(self):  # windows per core
        return self.WD * self.WHL * self.WWN

    @property
    def NT(self):    # 128-row tiles of local windows
        return self.NLOC // 128


FULL = Cfg()

_BUILD_CACHE: dict = {}


def mkap(base, extra_off, dims):
    """Manual AP on the same tensor; offsets/steps in flat element units
    (partition pitch == free size)."""
    return bass.AP(tensor=base.tensor, offset=base.offset + extra_off, ap=dims)


def build_module(cfg: Cfg):
    D, HL, HZ, DZ = cfg.D, cfg.HL, cfg.HZ, cfg.DZ
    WD, WHL, WWN, NLOC, NT = cfg.WD, cfg.WHL, cfg.WWN, cfg.NLOC, cfg.NT
    WZP = W + 2
    PHZ = DZ * HZ           # per-o plane size in Z storage

    nc = bacc.Bacc("TRN2", target_bir_lowering=False, debug=False,
                   num_devices=NCORES)

    # ---------------- I/O ----------------
    xt = nc.dram_tensor("xt", [C, D, HZ, W], BF16, kind="ExternalInput").ap()
    xr = nc.dram_tensor("xr", [D, HL, W, C], BF16, kind="ExternalInput").ap()
    w27 = nc.dram_tensor("w27", [C, 27], BF16, kind="ExternalInput").ap()
    dwcb = nc.dram_tensor("dwcb", [1, 1], F32, kind="ExternalInput").ap()
    centt = nc.dram_tensor("centt", [FEAD, K], F32, kind="ExternalInput").ap()
    qwt = nc.dram_tensor("qwt", [FEAD, FEAD], F32, kind="ExternalInput").ap()
    qb = nc.dram_tensor("qb", [FEAD, 1], F32, kind="ExternalInput").ap()
    kvk = nc.dram_tensor("kvk", [FEAD + 1, FEAD], F32, kind="ExternalInput").ap()
    kvv = nc.dram_tensor("kvv", [FEAD + 1, FEAD], F32, kind="ExternalInput").ap()
    upw = nc.dram_tensor("upw", [27, C], BF16, kind="ExternalInput").ap()
    y = nc.dram_tensor("y", [D, HL, W, C], BF16, kind="ExternalOutput").ap()

    with tile.TileContext(nc) as tc, ExitStack() as ctx:
        _body(ctx, tc, cfg, xt, xr, w27, dwcb, centt, qwt, qb, kvk, kvv, upw, y)
    nc.compile()
    return nc


def _body(ctx, tc, cfg, xt, xr, w27, dwcb, centt, qwt, qb, kvk, kvv, upw, y):
    nc = tc.nc
    D, HL, HZ, DZ = cfg.D, cfg.HL, cfg.HZ, cfg.DZ
    WD, WHL, WWN, NLOC, NT = cfg.WD, cfg.WHL, cfg.WWN, cfg.NLOC, cfg.NT
    WZP = W + 2
    PHZ = DZ * HZ
    EDGE = WWN * WD * 16          # per-edge halo elements (ww, wd, pd, pw)

    consts = ctx.enter_context(tc.tile_pool(name="consts", bufs=1))
    small = ctx.enter_context(tc.tile_pool(name="small", bufs=1))
    dram = ctx.enter_context(tc.tile_pool(name="dram", bufs=1, space="DRAM"))
    psA = ctx.enter_context(tc.tile_pool(name="psA", bufs=4, space="PSUM"))

    # ---------------- constants ----------------
    w27s = consts.tile([C, 27], BF16)
    nc.sync.dma_start(w27s[:], w27[:])
    upws = consts.tile([27, C], BF16)
    nc.sync.dma_start(upws[:], upw[:])
    centts = consts.tile([FEAD, K], F32)
    nc.sync.dma_start(centts[:], centt[:])
    qwts = consts.tile([FEAD, FEAD], F32)
    nc.sync.dma_start(qwts[:], qwt[:])
    qbs = consts.tile([FEAD, 1], F32)
    nc.sync.dma_start(qbs[:], qb[:])
    kvks = consts.tile([FEAD + 1, FEAD], F32)
    nc.sync.dma_start(kvks[:], kvk[:])
    kvvs = consts.tile([FEAD + 1, FEAD], F32)
    nc.sync.dma_start(kvvs[:], kvv[:])
    dwcb1 = consts.tile([1, 1], F32)
    nc.sync.dma_start(dwcb1[:], dwcb[:])
    dwcbb = consts.tile([W, 1], F32)
    nc.gpsimd.partition_broadcast(dwcbb[:], dwcb1[:])
    ident = consts.tile([FEAD, FEAD], F32)
    make_identity(nc, ident[:])

    # persistent mid-size tensors
    dnx = small.tile([W, P, P, WD, WHL], F32)        # (w; pd, ph, wd, wh)
    feat = small.tile([FEAD, NLOC], F32)             # fea^T (j, n) n=(ww,wd,wh)
    fa = small.tile([128, NT, FEAD + 1], F32)        # fea (n, j | 1)
    ee = small.tile([128, NT, K], F32)               # exp(logits) (n, k)
    sq = small.tile([FEAD, NLOC], F32)               # scratch for Square
    ov = small.tile([DZ, HZ, WZP], BF16)             # out volume (d; h, w)
    WQ2 = W // P

    # DRAM bounce buffers
    ar1_in = dram.tile([FEAD, 1], F32)
    ar1_out = dram.tile([FEAD, 1], F32, addr_space="Shared")
    ar2_in = dram.tile([K, FEAD + 1], F32)
    ar2_out = dram.tile([K, FEAD + 1], F32, addr_space="Shared")
    ovd = dram.tile([NLOC, FEAD], BF16)
    ovd2 = dram.tile([WHL, WD, P, P, W], BF16)  # (wh, wd, pd, ph, w) true-w order
    ag_in = dram.tile([2, EDGE], BF16)
    ag_out = dram.tile([NCORES, 2, EDGE], BF16, addr_space="Shared")

    # ================= PHASE A: dwc conv =================
    with tc.tile_pool(name="xin", bufs=3) as xpool, \
         tc.tile_pool(name="zps", bufs=4, space="PSUM") as zps, \
         tc.tile_pool(name="zsb", bufs=1) as zpool:

        zsb = zpool.tile([W, 9, 3, DZ, HZ], F32)     # Z (w; g, ow, dz, hz)
        zp = zpool.tile([W, 9, DZ, HZ], F32)         # Z shifted w+1 (ow=+1 taps)
        zm = zpool.tile([W, 9, DZ, HZ], F32)         # Z shifted w-1 (ow=-1 taps)

        # boundary zeros: d-halo planes of Z; full zero-init of the shifted
        # copies (covers their w-edge and d-halo rows in one go)
        nc.vector.memset(zsb[:, :, :, 0, :], 0.0)
        nc.vector.memset(zsb[:, :, :, DZ - 1, :], 0.0)
        nc.vector.memset(zp[:], 0.0)
        nc.vector.memset(zm[:], 0.0)

        for d in range(D):
            xin = xpool.tile([C, HZ, W], BF16)
            nc.sync.dma_start(xin[:], xt[:, d, :, :])
            for hb0 in range(0, HZ, 4):
                nr = min(4, HZ - hb0)
                ps = zps.tile([W, 4 * 27], F32, tag="zps")
                for i in range(nr):
                    nc.tensor.matmul(ps[:, i * 27:(i + 1) * 27],
                                     lhsT=xin[:, hb0 + i, :], rhs=w27s[:],
                                     start=True, stop=True)
                # copy psum -> zsb (o-major planes), strided dest
                src = ps[:, 0:nr * 27].rearrange("p (h g w3) -> p h g w3",
                                                 g=9, w3=3)
                dst = mkap(zsb[:], (d + 1) * HZ + hb0,
                           [[9 * 3 * PHZ, W], [1, nr], [3 * PHZ, 9], [PHZ, 3]])
                if (d * 5 + hb0 // 4) % 2 == 0:
                    nc.scalar.copy(dst, src)
                else:
                    nc.vector.tensor_copy(dst, src)

        # w-shifted copies.  Partitions are in permuted order w' = pw*32+ww
        # (true w = 4*ww+pw), so a +-1 shift in true w becomes two
        # consecutive-partition-range copies (+-32, and the pw wraparound).
        WQ = W // P   # 32
        for half in range(2):
            dz0 = 1 + half * (D // 2)
            ndz = D // 2
            span = [1, ndz * HZ]

            def shcopy(dstt, dst_fsz, dst_p0, nparts, src_p0, owi):
                nc.sync.dma_start(
                    out=mkap(dstt, dst_p0 * 9 * PHZ + dz0 * HZ,
                             [[9 * PHZ, nparts], [PHZ, 9], list(span)]),
                    in_=mkap(zsb[:], src_p0 * 27 * PHZ + owi * PHZ + dz0 * HZ,
                             [[27 * PHZ, nparts], [3 * PHZ, 9], list(span)]))

            # zp[p] = Z[w(p)+1]: dest [0,3*WQ) <- src [WQ,4*WQ);
            #                    dest [3*WQ, 4*WQ-1) <- src [1, WQ)
            shcopy(zp[:], 9 * PHZ, 0, 3 * WQ, WQ, 2)
            shcopy(zp[:], 9 * PHZ, 3 * WQ, WQ - 1, 1, 2)
            # zm[p] = Z[w(p)-1]: dest [WQ,4*WQ) <- src [0,3*WQ);
            #                    dest [1, WQ) <- src [3*WQ, 4*WQ-1)
            shcopy(zm[:], 9 * PHZ, WQ, 3 * WQ, 0, 0)
            shcopy(zm[:], 9 * PHZ, 1, WQ - 1, 3 * WQ, 0)

        # shifted-sum -> dnx, split per pd so every DVE op has <=3 free dims
        def zterm(g, ow, pd):
            od, oh = g // 3 - 1, g % 3 - 1
            if ow == 1:
                base, plane = zsb[:], (g * 3 + 1) * PHZ
            elif ow == 2:
                base, plane = zp[:], g * PHZ
            else:
                base, plane = zm[:], g * PHZ
            off = plane + (pd + od + 1) * HZ + (oh + 1)
            return mkap(base, off,
                        [[base.ap[0][0], W], [1, P], [4 * HZ, WD], [P, WHL]])

        for pd in range(P):
            acc = dnx[:, pd, :, :, :]
            nc.vector.tensor_add(acc, zterm(0, 0, pd), zterm(0, 1, pd))
            for g in range(9):
                for ow in range(3):
                    if g == 0 and ow in (0, 1):
                        continue
                    nc.vector.tensor_add(acc, acc, zterm(g, ow, pd))
        # + dwc bias (flat 2-D view; per-partition scalar broadcast)
        nc.vector.tensor_scalar_add(
            dnx[:].rearrange("p a b c d -> p (a b c d)"),
            dnx[:].rearrange("p a b c d -> p (a b c d)"), dwcbb[:])

    # ================= window partition =================
    # feat[j, n]; n = ww*(WD*WHL) + wd*WHL + wh ; j = pd*16 + ph*4 + pw
    # batched: one DMA per (pd, ph) covering the 4 pw partitions
    NWIN_D = WD * WHL
    fsz = P * P * NWIN_D                # free size of dnx
    for pd in range(P):
        for ph in range(P):
            j0 = pd * 16 + ph * 4
            off = pd * (P * NWIN_D) + ph * NWIN_D
            src = mkap(dnx[:], off,
                       [[WQ2 * fsz, P], [fsz, WWN], [1, NWIN_D]])
            nc.sync.dma_start(out=feat[j0:j0 + 4, :], in_=src)

    # ---- sumsq -> AllReduce -> rnorm ----
    su = small.tile([FEAD, 1], F32)
    nc.scalar.activation(sq[:], feat[:], AF.Square, accum_out=su[:])
    nc.sync.dma_start(ar1_in[:], su[:])
    nc.gpsimd.collective_compute("AllReduce", ALU.add,
                                 replica_groups=[list(range(NCORES))],
                                 ins=[ar1_in.opt()], outs=[ar1_out.opt()])
    rn = small.tile([FEAD, 1], F32)
    nc.sync.dma_start(rn[:], ar1_out[:])
    nc.scalar.sqrt(rn[:], rn[:])
    nc.vector.tensor_scalar_max(rn[:], rn[:], 1e-12)
    nc.vector.reciprocal(rn[:], rn[:])

    # fea (n-major) tiles + ones column, via PE transpose
    nc.vector.memset(fa[:, :, FEAD:FEAD + 1], 1.0)
    for t in range(NT):
        pt = psA.tile([128, FEAD], F32, tag="pb")
        nc.tensor.transpose(pt[:], feat[:, t * 128:(t + 1) * 128], ident[:])
        nc.scalar.copy(fa[:, t, 0:FEAD], pt[:])

    # ---- logits, exp, A matrix ----
    cst = small.tile([FEAD, K], F32)
    nc.vector.tensor_scalar_mul(cst[:], centts[:], rn[:])
    for t in range(NT):
        lg = psA.tile([128, K], F32, tag="pb")
        nc.tensor.matmul(lg[:], lhsT=feat[:, t * 128:(t + 1) * 128],
                         rhs=cst[:], start=True, stop=True)
        nc.scalar.activation(ee[:, t, :], lg[:], AF.Exp)

    aps = psA.tile([K, FEAD + 1], F32, tag="pb")
    for t in range(NT):
        nc.tensor.matmul(aps[:], lhsT=ee[:, t, :], rhs=fa[:, t, :],
                         start=(t == 0), stop=(t == NT - 1))
    asb = small.tile([K, FEAD + 1], F32)
    nc.scalar.copy(asb[:], aps[:])
    nc.sync.dma_start(ar2_in[:], asb[:])
    nc.gpsimd.collective_compute("AllReduce", ALU.add,
                                 replica_groups=[list(range(NCORES))],
                                 ins=[ar2_in.opt()], outs=[ar2_out.opt()])
    ag = small.tile([K, FEAD + 1], F32)
    nc.sync.dma_start(ag[:], ar2_out[:])

    # ---- new centroids, k/v, q ----
    rs = small.tile([K, 1], F32)
    nc.vector.reciprocal(rs[:], ag[:, FEAD:FEAD + 1])
    nc.vector.tensor_scalar_mul(ag[:, 0:FEAD], ag[:, 0:FEAD], rs[:])
    nct = small.tile([FEAD + 1, K], F32)
    ncp = psA.tile([FEAD, K], F32, tag="pb")
    nc.tensor.transpose(ncp[:], ag[:, 0:FEAD], ident[:])
    nc.vector.tensor_scalar_mul(nct[0:FEAD, :], ncp[:], rn[:])
    nc.vector.memset(nct[FEAD:FEAD + 1, :], 1.0)

    kt = small.tile([FEAD, K], F32)
    kp = psA.tile([FEAD, K], F32, tag="pb")
    nc.tensor.matmul(kp[:], lhsT=kvks[:], rhs=nct[:], start=True, stop=True)
    nc.scalar.copy(kt[:], kp[:])

    va = small.tile([K, FEAD + 1], F32)
    vp = psA.tile([K, FEAD], F32, tag="pb")
    nc.tensor.matmul(vp[:], lhsT=nct[:], rhs=kvvs[:], start=True, stop=True)
    nc.scalar.copy(va[:, 0:FEAD], vp[:])
    nc.vector.memset(va[:, FEAD:FEAD + 1], 1.0)

    qws = small.tile([FEAD, FEAD], F32)
    nc.vector.tensor_scalar_mul(qws[:], qwts[:], rn[:])
    qt = small.tile([FEAD, NLOC], F32)
    e2 = small.tile([K, NLOC], F32)
    CH = 512 if NLOC % 512 == 0 else 128
    for h0 in range(0, NLOC, CH):
        qp = psA.tile([FEAD, CH], F32, tag="pb")
        nc.tensor.matmul(qp[:], lhsT=qws[:], rhs=feat[:, h0:h0 + CH],
                         start=True, stop=True)
        nc.scalar.activation(qt[:, h0:h0 + CH], qp[:], AF.Identity, bias=qbs[:])
    for h0 in range(0, NLOC, CH):
        qk = psA.tile([K, CH], F32, tag="pb")
        nc.tensor.matmul(qk[:], lhsT=kt[:], rhs=qt[:, h0:h0 + CH],
                         start=True, stop=True)
        nc.scalar.activation(e2[:, h0:h0 + CH], qk[:], AF.Exp,
                             scale=float(FEAD) ** -0.5)

    # ---- attention output tiles -> DRAM dump + halo faces ----
    for t in range(NT):
        op = psA.tile([128, FEAD + 1], F32, tag="pb")
        nc.tensor.matmul(op[:], lhsT=e2[:, t * 128:(t + 1) * 128], rhs=va[:],
                         start=True, stop=True)
        rc = small.tile([128, 1], F32, tag="rc")
        nc.vector.reciprocal(rc[:], op[:, FEAD:FEAD + 1])
        ot = small.tile([128, FEAD], BF16, tag="ot")
        nc.vector.tensor_scalar_mul(ot[:], op[:, 0:FEAD], rc[:])
        nc.sync.dma_start(ovd[t * 128:(t + 1) * 128, :], ot[:])

    # reorder dump (n, j) -> volume-row layout (wh, wd, pd, ph, w) in DRAM
    for wh in range(WHL):
        for pd in range(P):
            for ph in range(P):
                nc.sync.dma_start(
                    out=mkap(ovd2[:],
                             ((wh * WD) * 16 + pd * P + ph) * W,
                             [[16 * W, WD], [P, WWN], [1, P]]),
                    in_=mkap(ovd[:],
                             wh * FEAD + pd * 16 + ph * P,
                             [[WHL * FEAD, WD], [NWIN_D * FEAD, WWN], [1, P]]))
    # halo faces (one contiguous-w call per edge); layout [edge][wd][pd][w]
    for edge in range(2):
        wh_e = 0 if edge == 0 else WHL - 1
        ph_e = 0 if edge == 0 else P - 1
        nc.sync.dma_start(
            out=mkap(ag_in[:], edge * EDGE, [[2 * EDGE, 1], [1, EDGE]]),
            in_=mkap(ovd2[:], ((wh_e * WD) * 16 + ph_e) * W,
                     [[16 * W, WD], [P * W, P], [1, W]]))

    nc.gpsimd.collective_compute("AllGather", ALU.bypass,
                                 replica_groups=[list(range(NCORES))],
                                 ins=[ag_in.opt()], outs=[ag_out.opt()])

    # ---- build ov (bf16): one DMA per (pd, ph) covering all (wd, wh, w) ----
    nc.vector.memset(ov[:], 0.0)
    OVF = HZ * WZP
    for pd in range(P):
        for ph in range(P):
            dst = mkap(ov[:], (pd + 1) * OVF + (ph + 1) * WZP + 1,
                       [[4 * OVF, WD], [P * WZP, WHL], [1, W]])
            src = mkap(ovd2[:], (pd * P + ph) * W,
                       [[16 * W, WD], [WD * 16 * W, WHL], [1, W]])
            nc.sync.dma_start(out=dst, in_=src)
    # halo rows from AllGather (dynamic rank offsets, edge cores skip);
    # one batched DMA per edge over (wd, pd, w)
    pid = nc.partition_id()
    dstl = mkap(ov[:], 1 * OVF + 0 * WZP + 1,
                [[4 * OVF, WD], [OVF, P], [1, W]])
    srcl = mkap(ag_out[:], (pid - 1) * (2 * EDGE) + 1 * EDGE,
                [[P * W, WD], [W, P], [1, W]])
    nc.gpsimd.dma_start(out=dstl, in_=srcl, cond=(pid >= 1))
    dsth = mkap(ov[:], 1 * OVF + (HZ - 1) * WZP + 1,
                [[4 * OVF, WD], [OVF, P], [1, W]])
    srch = mkap(ag_out[:], (pid + 1) * (2 * EDGE) + 0 * EDGE,
                [[P * W, WD], [W, P], [1, W]])
    nc.gpsimd.dma_start(out=dsth, in_=srch, cond=(pid <= NCORES - 2))

    # ================= PHASE C: upc conv + residual =================
    # im2col built ONCE for the whole volume (27 DMAs); per-d batched
    # bf16 residual loads and y stores (32 + 32 DMAs).
    with tc.tile_pool(name="i2c", bufs=1) as cpool, \
         tc.tile_pool(name="xrp", bufs=4) as xrp, \
         tc.tile_pool(name="yp", bufs=4) as yp, \
         tc.tile_pool(name="psC", bufs=4, space="PSUM") as psC:
        i2c = cpool.tile([27, D * HL * W], BF16)
        for o in range(27):
            od, oh, ow = o // 9 - 1, (o // 3) % 3 - 1, o % 3 - 1
            src = mkap(ov[:], (od + 1) * OVF + (oh + 1) * WZP + (ow + 1),
                       [[OVF, D], [WZP, HL], [1, W]])
            nc.sync.dma_start(out=i2c[o:o + 1, :], in_=src)
        for d in range(D):
            xrd = xrp.tile([W, HL, C], BF16)
            nc.sync.dma_start(
                out=xrd[:],
                in_=mkap(xr[:], d * HL * W * C,
                         [[C, W], [W * C, HL], [1, C]]))
            ysd = yp.tile([W, HL, C], BF16)
            for hq in range(HL // 4):
                psc = psC.tile([W, 4 * C], F32, tag="psc")
                for i in range(4):
                    h = hq * 4 + i
                    nc.tensor.matmul(
                        psc[:, i * C:(i + 1) * C],
                        lhsT=i2c[:, (d * HL + h) * W:(d * HL + h + 1) * W],
                        rhs=upws[:], start=True, stop=True)
                nc.vector.tensor_add(
                    ysd[:, hq * 4:(hq + 1) * 4, :],
                    psc[:].rearrange("p (a b) -> p a b", a=4),
                    xrd[:, hq * 4:(hq + 1) * 4, :])
            nc.sync.dma_start(
                out=mkap(y[:], d * HL * W * C,
                         [[C, W], [W * C, HL], [1, C]]),
                in_=ysd[:])


# ======================= host side =======================

def _prep_inputs(cfg: Cfg, inputs):
    x = np.asarray(inputs["x"], np.float32)[0]          # (D, H, W, C)
    D, H = cfg.D, NCORES * cfg.HL
    assert x.shape == (D, H, W, C), x.shape
    HLp = cfg.HL

    xpad = np.zeros((D, H + 2, W, C), np.float32)
    xpad[:, 1:H + 1] = x
    dwc_w = np.asarray(inputs["dwc_w"], np.float32)
    upc_w = np.asarray(inputs["upc_w"], np.float32)
    q_w = np.asarray(inputs["q_w"], np.float32)
    kv_w = np.asarray(inputs["kv_w"], np.float32)
    q_b = np.asarray(inputs["q_b"], np.float32)
    kv_b = np.asarray(inputs["kv_b"], np.float32)
    upc_b = np.asarray(inputs["upc_b"], np.float32)
    cent = np.asarray(inputs["centroids"], np.float32)

    w27 = np.ascontiguousarray(dwc_w[0].reshape(C, 27)).astype(ml_dtypes.bfloat16)
    upwt = np.ascontiguousarray(upc_w[:, 0].reshape(C, 27).T).astype(ml_dtypes.bfloat16)
    dwcb = np.asarray(inputs["dwc_b"], np.float32).reshape(1, 1)
    centT = np.ascontiguousarray(cent.T)
    qwT = np.ascontiguousarray(q_w.T)
    qbv = q_b.reshape(FEAD, 1)
    kvk = np.concatenate([kv_w[0:FEAD].T, kv_b[None, 0:FEAD]], 0)
    kvv = np.concatenate([kv_w[FEAD:2 * FEAD].T, kv_b[None, FEAD:2 * FEAD]], 0)
    kvk = np.ascontiguousarray(kvk)
    kvv = np.ascontiguousarray(kvv)

    in_maps = []
    for k in range(NCORES):
        h0 = k * HLp
        slab = xpad[:, h0:h0 + HLp + 2]                  # (D, HZ, W, C)
        worder = np.array([4 * (p % 32) + p // 32 for p in range(W)])
        slab = slab[:, :, worder, :]
        xt = np.ascontiguousarray(slab.transpose(3, 0, 1, 2)).astype(
            ml_dtypes.bfloat16)
        xrs = np.ascontiguousarray(x[:, h0:h0 + HLp]) + upc_b[None, None, None, :]
        in_maps.append({
            "xt": xt, "xr": xrs.astype(ml_dtypes.bfloat16), "w27": w27,
            "dwcb": dwcb,
            "centt": centT, "qwt": qwT, "qb": qbv, "kvk": kvk, "kvv": kvv,
            "upw": upwt,
        })
    return in_maps


def _get_built(cfg: Cfg):
    if cfg not in _BUILD_CACHE:
        _BUILD_CACHE[cfg] = build_module(cfg)
    return _BUILD_CACHE[cfg]


def _postprocess(cfg: Cfg, res):
    ys = [res.results[k]["y"] for k in range(NCORES)]
    yfull = np.concatenate(ys, axis=1)                   # (D, H, W, C)
    return yfull[None].astype(np.float32)


def kernel(**inputs):
    cfg = FULL
    nc = _get_built(cfg)
    in_maps = _prep_inputs(cfg, inputs)
    res = bass_utils.run_bass_kernel_spmd(nc, in_maps,
                                          core_ids=list(range(NCORES)))
    return _postprocess(cfg, res)



# revision 20
# speedup vs baseline: 1.3009x; 1.0785x over previous
"""Trainium2 Bass kernel for nn_ClusterAttn (vq_codebook).

Strategy (8 NeuronCores, SPMD):
  - Shard the h axis (128) into 8 slabs of 16 rows; windows (4^3) stay core-local.
  - Host-side prep: per-core channel-major bf16 slab of x (with h halo) for the
    dwc conv; natural fp32 slab for the residual; pre-transposed weights.
  - Phase A (dwc conv): per (d, h-row) matmul  Z[w, o] = sum_c x[c,*] * w27[c, o]
    (o = 27 taps), then 27 free-dim-shifted DVE adds produce dnx in a
    window-friendly layout. w+-1 shifts are pre-materialized by two DMA copies.
  - Phase B (cluster attention): all matmuls on 64-wide fea tiles; softmax over
    windows needs two tiny AllReduces (sum of squares; exp-sums + soft counts).
    No max-subtraction (logits are tiny; validated numerically).
  - Phase C (upc conv + residual): out volume -> DRAM bounce -> ov tile (d on
    partitions) -> 27-row im2col built by DMA -> one matmul per (d,h) row,
    + fp32 residual, streamed back to HBM.  h-halo of ov comes from an
    AllGather of window-face slices.
"""

import os
import sys
from contextlib import ExitStack
from dataclasses import dataclass

import numpy as np

for _p in ("/opt/trn_rl_repo",):
    if os.path.isdir(_p) and _p not in sys.path:
        sys.path.insert(0, _p)

os.environ.setdefault("MYCRO_LOCAL_CACHE", "1")

import ml_dtypes  # noqa: E402
import concourse.bass as bass  # noqa: E402
import concourse.tile as tile  # noqa: E402
from concourse import bacc, mybir  # noqa: E402
from concourse import bass_utils  # noqa: E402
from concourse.masks import make_identity  # noqa: E402

F32 = mybir.dt.float32
BF16 = mybir.dt.bfloat16
AF = mybir.ActivationFunctionType
ALU = mybir.AluOpType

C = 96          # channels
P = 4           # window edge
FEAD = 64       # P^3
K = 64          # clusters
NCORES = 8
W = 128         # w extent == partition count


@dataclass(frozen=True)
class Cfg:
    D: int = 32      # d extent
    HL: int = 16     # h rows per core (total H = 8*HL)

    @property
    def HZ(self):     # h rows incl halo
        return self.HL + 2

    @property
    def DZ(self):     # d extent incl halo
        return self.D + 2

    @property
    def WD(self):
        return self.D // P

    @property
    def WHL(self):
        return self.HL // P

    @property
    def WWN(self):
        return W // P

    @property
    def NLOC(self):  # windows per core
        return self.WD * self.WHL * self.WWN

    @property
    def NT(self):    # 128-row tiles of local windows
        return self.NLOC // 128


FULL = Cfg()

_BUILD_CACHE: dict = {}


def mkap(base, extra_off, dims):
    """Manual AP on the same tensor; offsets/steps in flat element units
    (partition pitch == free size)."""
    return bass.AP(tensor=base.tensor, offset=base.offset + extra_off, ap=dims)


def build_module(cfg: Cfg):
    D, HL, HZ, DZ = cfg.D, cfg.HL, cfg.HZ, cfg.DZ
    WD, WHL, WWN, NLOC, NT = cfg.WD, cfg.WHL, cfg.WWN, cfg.NLOC, cfg.NT
    WZP = W + 2
    PHZ = DZ * HZ           # per-o plane size in Z storage

    nc = bacc.Bacc("TRN2", target_bir_lowering=False, debug=False,
                   num_devices=NCORES)

    # ---------------- I/O ----------------
    xt = nc.dram_tensor("xt", [C, D, HZ, W], BF16, kind="ExternalInput").ap()
    xr = nc.dram_tensor("xr", [W, D, HL, C], BF16, kind="ExternalInput").ap()
    w27 = nc.dram_tensor("w27", [C, 27], BF16, kind="ExternalInput").ap()
    dwcb = nc.dram_tensor("dwcb", [1, 1], F32, kind="ExternalInput").ap()
    centt = nc.dram_tensor("centt", [FEAD, K], F32, kind="ExternalInput").ap()
    qwt = nc.dram_tensor("qwt", [FEAD, FEAD], F32, kind="ExternalInput").ap()
    qb = nc.dram_tensor("qb", [FEAD, 1], F32, kind="ExternalInput").ap()
    kvk = nc.dram_tensor("kvk", [FEAD + 1, FEAD], F32, kind="ExternalInput").ap()
    kvv = nc.dram_tensor("kvv", [FEAD + 1, FEAD], F32, kind="ExternalInput").ap()
    upw = nc.dram_tensor("upw", [27, C], BF16, kind="ExternalInput").ap()
    y = nc.dram_tensor("y", [W, D, HL, C], BF16, kind="ExternalOutput").ap()

    with tile.TileContext(nc) as tc, ExitStack() as ctx:
        _body(ctx, tc, cfg, xt, xr, w27, dwcb, centt, qwt, qb, kvk, kvv, upw, y)
    nc.compile()
    return nc


def _body(ctx, tc, cfg, xt, xr, w27, dwcb, centt, qwt, qb, kvk, kvv, upw, y):
    nc = tc.nc
    D, HL, HZ, DZ = cfg.D, cfg.HL, cfg.HZ, cfg.DZ
    WD, WHL, WWN, NLOC, NT = cfg.WD, cfg.WHL, cfg.WWN, cfg.NLOC, cfg.NT
    WZP = W + 2
    PHZ = DZ * HZ
    EDGE = WWN * WD * 16          # per-edge halo elements (ww, wd, pd, pw)

    consts = ctx.enter_context(tc.tile_pool(name="consts", bufs=1))
    small = ctx.enter_context(tc.tile_pool(name="small", bufs=1))
    dram = ctx.enter_context(tc.tile_pool(name="dram", bufs=1, space="DRAM"))
    psA = ctx.enter_context(tc.tile_pool(name="psA", bufs=4, space="PSUM"))

    # ---------------- constants ----------------
    w27s = consts.tile([C, 27], BF16)
    nc.sync.dma_start(w27s[:], w27[:])
    upws = consts.tile([27, C], BF16)
    nc.sync.dma_start(upws[:], upw[:])
    centts = consts.tile([FEAD, K], F32)
    nc.sync.dma_start(centts[:], centt[:])
    qwts = consts.tile([FEAD, FEAD], F32)
    nc.sync.dma_start(qwts[:], qwt[:])
    qbs = consts.tile([FEAD, 1], F32)
    nc.sync.dma_start(qbs[:], qb[:])
    kvks = consts.tile([FEAD + 1, FEAD], F32)
    nc.sync.dma_start(kvks[:], kvk[:])
    kvvs = consts.tile([FEAD + 1, FEAD], F32)
    nc.sync.dma_start(kvvs[:], kvv[:])
    dwcb1 = consts.tile([1, 1], F32)
    nc.sync.dma_start(dwcb1[:], dwcb[:])
    dwcbb = consts.tile([W, 1], F32)
    nc.gpsimd.partition_broadcast(dwcbb[:], dwcb1[:])
    ident = consts.tile([FEAD, FEAD], F32)
    make_identity(nc, ident[:])

    # persistent mid-size tensors
    dnx = small.tile([W, P, P, WD, WHL], F32)        # (w; pd, ph, wd, wh)
    feat = small.tile([FEAD, NLOC], F32)             # fea^T (j, n) n=(ww,wd,wh)
    fa = small.tile([128, NT, FEAD + 1], F32)        # fea (n, j | 1)
    ee = small.tile([128, NT, K], F32)               # exp(logits) (n, k)
    sq = small.tile([FEAD, NLOC], F32)               # scratch for Square
    ov = small.tile([DZ, HZ, WZP], BF16)             # out volume (d; h, w)
    WQ2 = W // P

    # DRAM bounce buffers
    ar1_in = dram.tile([FEAD, 1], F32)
    ar1_out = dram.tile([FEAD, 1], F32, addr_space="Shared")
    ar2_in = dram.tile([K, FEAD + 1], F32)
    ar2_out = dram.tile([K, FEAD + 1], F32, addr_space="Shared")
    ovd = dram.tile([NLOC, FEAD], BF16)
    ovd2 = dram.tile([WHL, WD, P, P, W], BF16)  # (wh, wd, pd, ph, w) true-w order
    ag_in = dram.tile([2, EDGE], BF16)
    ag_out = dram.tile([NCORES, 2, EDGE], BF16, addr_space="Shared")

    # ================= PHASE A: dwc conv =================
    with tc.tile_pool(name="xin", bufs=3) as xpool, \
         tc.tile_pool(name="zps", bufs=4, space="PSUM") as zps, \
         tc.tile_pool(name="zsb", bufs=1) as zpool:

        zsb = zpool.tile([W, 9, 3, DZ, HZ], F32)     # Z (w; g, ow, dz, hz)
        zp = zpool.tile([W, 9, DZ, HZ], F32)         # Z shifted w+1 (ow=+1 taps)
        zm = zpool.tile([W, 9, DZ, HZ], F32)         # Z shifted w-1 (ow=-1 taps)

        # boundary zeros: d-halo planes of Z; full zero-init of the shifted
        # copies (covers their w-edge and d-halo rows in one go)
        nc.vector.memset(zsb[:, :, :, 0, :], 0.0)
        nc.vector.memset(zsb[:, :, :, DZ - 1, :], 0.0)
        nc.vector.memset(zp[:], 0.0)
        nc.vector.memset(zm[:], 0.0)

        for d in range(D):
            xin = xpool.tile([C, HZ, W], BF16)
            nc.sync.dma_start(xin[:], xt[:, d, :, :])
            for hb0 in range(0, HZ, 4):
                nr = min(4, HZ - hb0)
                ps = zps.tile([W, 4 * 27], F32, tag="zps")
                for i in range(nr):
                    nc.tensor.matmul(ps[:, i * 27:(i + 1) * 27],
                                     lhsT=xin[:, hb0 + i, :], rhs=w27s[:],
                                     start=True, stop=True)
                # copy psum -> zsb (o-major planes), strided dest
                src = ps[:, 0:nr * 27].rearrange("p (h g w3) -> p h g w3",
                                                 g=9, w3=3)
                dst = mkap(zsb[:], (d + 1) * HZ + hb0,
                           [[9 * 3 * PHZ, W], [1, nr], [3 * PHZ, 9], [PHZ, 3]])
                if (d * 5 + hb0 // 4) % 2 == 0:
                    nc.scalar.copy(dst, src)
                else:
                    nc.vector.tensor_copy(dst, src)

        # w-shifted copies.  Partitions are in permuted order w' = pw*32+ww
        # (true w = 4*ww+pw), so a +-1 shift in true w becomes two
        # consecutive-partition-range copies (+-32, and the pw wraparound).
        WQ = W // P   # 32
        for half in range(2):
            dz0 = 1 + half * (D // 2)
            ndz = D // 2
            span = [1, ndz * HZ]

            def shcopy(dstt, dst_fsz, dst_p0, nparts, src_p0, owi):
                nc.sync.dma_start(
                    out=mkap(dstt, dst_p0 * 9 * PHZ + dz0 * HZ,
                             [[9 * PHZ, nparts], [PHZ, 9], list(span)]),
                    in_=mkap(zsb[:], src_p0 * 27 * PHZ + owi * PHZ + dz0 * HZ,
                             [[27 * PHZ, nparts], [3 * PHZ, 9], list(span)]))

            # zp[p] = Z[w(p)+1]: dest [0,3*WQ) <- src [WQ,4*WQ);
            #                    dest [3*WQ, 4*WQ-1) <- src [1, WQ)
            shcopy(zp[:], 9 * PHZ, 0, 3 * WQ, WQ, 2)
            shcopy(zp[:], 9 * PHZ, 3 * WQ, WQ - 1, 1, 2)
            # zm[p] = Z[w(p)-1]: dest [WQ,4*WQ) <- src [0,3*WQ);
            #                    dest [1, WQ) <- src [3*WQ, 4*WQ-1)
            shcopy(zm[:], 9 * PHZ, WQ, 3 * WQ, 0, 0)
            shcopy(zm[:], 9 * PHZ, 1, WQ - 1, 3 * WQ, 0)

        # shifted-sum -> dnx, split per pd so every DVE op has <=3 free dims
        def zterm(g, ow, pd):
            od, oh = g // 3 - 1, g % 3 - 1
            if ow == 1:
                base, plane = zsb[:], (g * 3 + 1) * PHZ
            elif ow == 2:
                base, plane = zp[:], g * PHZ
            else:
                base, plane = zm[:], g * PHZ
            off = plane + (pd + od + 1) * HZ + (oh + 1)
            return mkap(base, off,
                        [[base.ap[0][0], W], [1, P], [4 * HZ, WD], [P, WHL]])

        for pd in range(P):
            acc = dnx[:, pd, :, :, :]
            nc.vector.tensor_add(acc, zterm(0, 0, pd), zterm(0, 1, pd))
            for g in range(9):
                for ow in range(3):
                    if g == 0 and ow in (0, 1):
                        continue
                    nc.vector.tensor_add(acc, acc, zterm(g, ow, pd))
        # + dwc bias (flat 2-D view; per-partition scalar broadcast)
        nc.vector.tensor_scalar_add(
            dnx[:].rearrange("p a b c d -> p (a b c d)"),
            dnx[:].rearrange("p a b c d -> p (a b c d)"), dwcbb[:])

    # ================= window partition =================
    # feat[j, n]; n = ww*(WD*WHL) + wd*WHL + wh ; j = pd*16 + ph*4 + pw
    # batched: one DMA per (pd, ph) covering the 4 pw partitions
    NWIN_D = WD * WHL
    fsz = P * P * NWIN_D                # free size of dnx
    for pd in range(P):
        for ph in range(P):
            j0 = pd * 16 + ph * 4
            off = pd * (P * NWIN_D) + ph * NWIN_D
            src = mkap(dnx[:], off,
                       [[WQ2 * fsz, P], [fsz, WWN], [1, NWIN_D]])
            nc.sync.dma_start(out=feat[j0:j0 + 4, :], in_=src)

    # ---- sumsq -> AllReduce -> rnorm ----
    su = small.tile([FEAD, 1], F32)
    nc.scalar.activation(sq[:], feat[:], AF.Square, accum_out=su[:])
    nc.sync.dma_start(ar1_in[:], su[:])
    nc.gpsimd.collective_compute("AllReduce", ALU.add,
                                 replica_groups=[list(range(NCORES))],
                                 ins=[ar1_in.opt()], outs=[ar1_out.opt()])
    rn = small.tile([FEAD, 1], F32)
    nc.sync.dma_start(rn[:], ar1_out[:])
    nc.scalar.sqrt(rn[:], rn[:])
    nc.vector.tensor_scalar_max(rn[:], rn[:], 1e-12)
    nc.vector.reciprocal(rn[:], rn[:])

    # fea (n-major) tiles + ones column, via PE transpose
    nc.vector.memset(fa[:, :, FEAD:FEAD + 1], 1.0)
    for t in range(NT):
        pt = psA.tile([128, FEAD], F32, tag="pb")
        nc.tensor.transpose(pt[:], feat[:, t * 128:(t + 1) * 128], ident[:])
        nc.scalar.copy(fa[:, t, 0:FEAD], pt[:])

    # ---- logits, exp, A matrix ----
    cst = small.tile([FEAD, K], F32)
    nc.vector.tensor_scalar_mul(cst[:], centts[:], rn[:])
    for t in range(NT):
        lg = psA.tile([128, K], F32, tag="pb")
        nc.tensor.matmul(lg[:], lhsT=feat[:, t * 128:(t + 1) * 128],
                         rhs=cst[:], start=True, stop=True)
        nc.scalar.activation(ee[:, t, :], lg[:], AF.Exp)

    aps = psA.tile([K, FEAD + 1], F32, tag="pb")
    for t in range(NT):
        nc.tensor.matmul(aps[:], lhsT=ee[:, t, :], rhs=fa[:, t, :],
                         start=(t == 0), stop=(t == NT - 1))
    asb = small.tile([K, FEAD + 1], F32)
    nc.scalar.copy(asb[:], aps[:])
    nc.sync.dma_start(ar2_in[:], asb[:])
    nc.gpsimd.collective_compute("AllReduce", ALU.add,
                                 replica_groups=[list(range(NCORES))],
                                 ins=[ar2_in.opt()], outs=[ar2_out.opt()])
    ag = small.tile([K, FEAD + 1], F32)
    nc.sync.dma_start(ag[:], ar2_out[:])

    # ---- new centroids, k/v, q ----
    rs = small.tile([K, 1], F32)
    nc.vector.reciprocal(rs[:], ag[:, FEAD:FEAD + 1])
    nc.vector.tensor_scalar_mul(ag[:, 0:FEAD], ag[:, 0:FEAD], rs[:])
    nct = small.tile([FEAD + 1, K], F32)
    ncp = psA.tile([FEAD, K], F32, tag="pb")
    nc.tensor.transpose(ncp[:], ag[:, 0:FEAD], ident[:])
    nc.vector.tensor_scalar_mul(nct[0:FEAD, :], ncp[:], rn[:])
    nc.vector.memset(nct[FEAD:FEAD + 1, :], 1.0)

    kt = small.tile([FEAD, K], F32)
    kp = psA.tile([FEAD, K], F32, tag="pb")
    nc.tensor.matmul(kp[:], lhsT=kvks[:], rhs=nct[:], start=True, stop=True)
    nc.scalar.copy(kt[:], kp[:])

    va = small.tile([K, FEAD + 1], F32)
    vp = psA.tile([K, FEAD], F32, tag="pb")
    nc.tensor.matmul(vp[:], lhsT=nct[:], rhs=kvvs[:], start=True, stop=True)
    nc.scalar.copy(va[:, 0:FEAD], vp[:])
    nc.vector.memset(va[:, FEAD:FEAD + 1], 1.0)

    qws = small.tile([FEAD, FEAD], F32)
    nc.vector.tensor_scalar_mul(qws[:], qwts[:], rn[:])
    qt = small.tile([FEAD, NLOC], F32)
    e2 = small.tile([K, NLOC], F32)
    CH = 512 if NLOC % 512 == 0 else 128
    for h0 in range(0, NLOC, CH):
        qp = psA.tile([FEAD, CH], F32, tag="pb")
        nc.tensor.matmul(qp[:], lhsT=qws[:], rhs=feat[:, h0:h0 + CH],
                         start=True, stop=True)
        nc.scalar.activation(qt[:, h0:h0 + CH], qp[:], AF.Identity, bias=qbs[:])
    for h0 in range(0, NLOC, CH):
        qk = psA.tile([K, CH], F32, tag="pb")
        nc.tensor.matmul(qk[:], lhsT=kt[:], rhs=qt[:, h0:h0 + CH],
                         start=True, stop=True)
        nc.scalar.activation(e2[:, h0:h0 + CH], qk[:], AF.Exp,
                             scale=float(FEAD) ** -0.5)

    # ---- attention output tiles -> DRAM dump + halo faces ----
    for t in range(NT):
        op = psA.tile([128, FEAD + 1], F32, tag="pb")
        nc.tensor.matmul(op[:], lhsT=e2[:, t * 128:(t + 1) * 128], rhs=va[:],
                         start=True, stop=True)
        rc = small.tile([128, 1], F32, tag="rc")
        nc.vector.reciprocal(rc[:], op[:, FEAD:FEAD + 1])
        ot = small.tile([128, FEAD], BF16, tag="ot")
        nc.vector.tensor_scalar_mul(ot[:], op[:, 0:FEAD], rc[:])
        nc.sync.dma_start(ovd[t * 128:(t + 1) * 128, :], ot[:])

    # reorder dump (n, j) -> volume-row layout (wh, wd, pd, ph, w) in DRAM
    for wh in range(WHL):
        for pd in range(P):
            for ph in range(P):
                nc.sync.dma_start(
                    out=mkap(ovd2[:],
                             ((wh * WD) * 16 + pd * P + ph) * W,
                             [[16 * W, WD], [P, WWN], [1, P]]),
                    in_=mkap(ovd[:],
                             wh * FEAD + pd * 16 + ph * P,
                             [[WHL * FEAD, WD], [NWIN_D * FEAD, WWN], [1, P]]))
    # halo faces (one contiguous-w call per edge); layout [edge][wd][pd][w]
    for edge in range(2):
        wh_e = 0 if edge == 0 else WHL - 1
        ph_e = 0 if edge == 0 else P - 1
        nc.sync.dma_start(
            out=mkap(ag_in[:], edge * EDGE, [[2 * EDGE, 1], [1, EDGE]]),
            in_=mkap(ovd2[:], ((wh_e * WD) * 16 + ph_e) * W,
                     [[16 * W, WD], [P * W, P], [1, W]]))

    nc.gpsimd.collective_compute("AllGather", ALU.bypass,
                                 replica_groups=[list(range(NCORES))],
                                 ins=[ag_in.opt()], outs=[ag_out.opt()])

    # ---- build ov (bf16): one DMA per (pd, ph) covering all (wd, wh, w) ----
    nc.vector.memset(ov[:], 0.0)
    OVF = HZ * WZP
    for pd in range(P):
        for ph in range(P):
            dst = mkap(ov[:], (pd + 1) * OVF + (ph + 1) * WZP + 1,
                       [[4 * OVF, WD], [P * WZP, WHL], [1, W]])
            src = mkap(ovd2[:], (pd * P + ph) * W,
                       [[16 * W, WD], [WD * 16 * W, WHL], [1, W]])
            nc.sync.dma_start(out=dst, in_=src)
    # halo rows from AllGather (dynamic rank offsets, edge cores skip);
    # one batched DMA per edge over (wd, pd, w)
    pid = nc.partition_id()
    dstl = mkap(ov[:], 1 * OVF + 0 * WZP + 1,
                [[4 * OVF, WD], [OVF, P], [1, W]])
    srcl = mkap(ag_out[:], (pid - 1) * (2 * EDGE) + 1 * EDGE,
                [[P * W, WD], [W, P], [1, W]])
    nc.gpsimd.dma_start(out=dstl, in_=srcl, cond=(pid >= 1))
    dsth = mkap(ov[:], 1 * OVF + (HZ - 1) * WZP + 1,
                [[4 * OVF, WD], [OVF, P], [1, W]])
    srch = mkap(ag_out[:], (pid + 1) * (2 * EDGE) + 0 * EDGE,
                [[P * W, WD], [W, P], [1, W]])
    nc.gpsimd.dma_start(out=dsth, in_=srch, cond=(pid <= NCORES - 2))

    # ================= PHASE C: upc conv + residual =================
    # im2col built ONCE for the whole volume (27 DMAs); per-d batched
    # bf16 residual loads and y stores (32 + 32 DMAs).
    with tc.tile_pool(name="i2c", bufs=1) as cpool, \
         tc.tile_pool(name="xrp", bufs=4) as xrp, \
         tc.tile_pool(name="yp", bufs=4) as yp, \
         tc.tile_pool(name="psC", bufs=4, space="PSUM") as psC:
        i2c = cpool.tile([27, D * HL * W], BF16)
        for o in range(27):
            od, oh, ow = o // 9 - 1, (o // 3) % 3 - 1, o % 3 - 1
            src = mkap(ov[:], (od + 1) * OVF + (oh + 1) * WZP + (ow + 1),
                       [[OVF, D], [WZP, HL], [1, W]])
            eng = nc.sync if o % 2 == 0 else nc.scalar
            eng.dma_start(out=i2c[o:o + 1, :], in_=src)
        for d in range(D):
            xrd = xrp.tile([W, HL, C], BF16)
            eng = nc.sync if d % 2 == 0 else nc.scalar
            eng.dma_start(
                out=xrd[:],
                in_=mkap(xr[:], d * HL * C,
                         [[D * HL * C, W], [1, HL * C]]))
            ysd = yp.tile([W, HL, C], BF16)
            for hq in range(HL // 4):
                psc = psC.tile([W, 4 * C], F32, tag="psc")
                for i in range(4):
                    h = hq * 4 + i
                    nc.tensor.matmul(
                        psc[:, i * C:(i + 1) * C],
                        lhsT=i2c[:, (d * HL + h) * W:(d * HL + h + 1) * W],
                        rhs=upws[:], start=True, stop=True)
                nc.vector.tensor_add(
                    ysd[:, hq * 4:(hq + 1) * 4, :],
                    psc[:].rearrange("p (a b) -> p a b", a=4),
                    xrd[:, hq * 4:(hq + 1) * 4, :])
            eng2 = nc.scalar if d % 2 == 0 else nc.sync
            eng2.dma_start(
                out=mkap(y[:], d * HL * C,
                         [[D * HL * C, W], [1, HL * C]]),
                in_=ysd[:])


# ======================= host side =======================

def _prep_inputs(cfg: Cfg, inputs):
    x = np.asarray(inputs["x"], np.float32)[0]          # (D, H, W, C)
    D, H = cfg.D, NCORES * cfg.HL
    assert x.shape == (D, H, W, C), x.shape
    HLp = cfg.HL

    xpad = np.zeros((D, H + 2, W, C), np.float32)
    xpad[:, 1:H + 1] = x
    dwc_w = np.asarray(inputs["dwc_w"], np.float32)
    upc_w = np.asarray(inputs["upc_w"], np.float32)
    q_w = np.asarray(inputs["q_w"], np.float32)
    kv_w = np.asarray(inputs["kv_w"], np.float32)
    q_b = np.asarray(inputs["q_b"], np.float32)
    kv_b = np.asarray(inputs["kv_b"], np.float32)
    upc_b = np.asarray(inputs["upc_b"], np.float32)
    cent = np.asarray(inputs["centroids"], np.float32)

    w27 = np.ascontiguousarray(dwc_w[0].reshape(C, 27)).astype(ml_dtypes.bfloat16)
    upwt = np.ascontiguousarray(upc_w[:, 0].reshape(C, 27).T).astype(ml_dtypes.bfloat16)
    dwcb = np.asarray(inputs["dwc_b"], np.float32).reshape(1, 1)
    centT = np.ascontiguousarray(cent.T)
    qwT = np.ascontiguousarray(q_w.T)
    qbv = q_b.reshape(FEAD, 1)
    kvk = np.concatenate([kv_w[0:FEAD].T, kv_b[None, 0:FEAD]], 0)
    kvv = np.concatenate([kv_w[FEAD:2 * FEAD].T, kv_b[None, FEAD:2 * FEAD]], 0)
    kvk = np.ascontiguousarray(kvk)
    kvv = np.ascontiguousarray(kvv)

    in_maps = []
    for k in range(NCORES):
        h0 = k * HLp
        slab = xpad[:, h0:h0 + HLp + 2]                  # (D, HZ, W, C)
        worder = np.array([4 * (p % 32) + p // 32 for p in range(W)])
        slab = slab[:, :, worder, :]
        xt = np.ascontiguousarray(slab.transpose(3, 0, 1, 2)).astype(
            ml_dtypes.bfloat16)
        xrs = x[:, h0:h0 + HLp] + upc_b[None, None, None, :]
        xrs = np.ascontiguousarray(xrs.transpose(2, 0, 1, 3))  # (W, D, HL, C)
        in_maps.append({
            "xt": xt, "xr": xrs.astype(ml_dtypes.bfloat16), "w27": w27,
            "dwcb": dwcb,
            "centt": centT, "qwt": qwT, "qb": qbv, "kvk": kvk, "kvv": kvv,
            "upw": upwt,
        })
    return in_maps


def _get_built(cfg: Cfg):
    if cfg not in _BUILD_CACHE:
        _BUILD_CACHE[cfg] = build_module(cfg)
    return _BUILD_CACHE[cfg]


def _postprocess(cfg: Cfg, res):
    # per-core y is (W, D, HL, C) bf16 -> (D, HL, W, C) f32, concat on h
    ys = [np.asarray(res.results[k]["y"]).astype(np.float32)
          .transpose(1, 2, 0, 3) for k in range(NCORES)]
    yfull = np.concatenate(ys, axis=1)                   # (D, H, W, C)
    return yfull[None]


def kernel(**inputs):
    cfg = FULL
    nc = _get_built(cfg)
    in_maps = _prep_inputs(cfg, inputs)
    res = bass_utils.run_bass_kernel_spmd(nc, in_maps,
                                          core_ids=list(range(NCORES)))
    return _postprocess(cfg, res)



# revision 27
# speedup vs baseline: 1.3259x; 1.0192x over previous
"""Trainium2 Bass kernel for nn_ClusterAttn (vq_codebook).

Strategy (8 NeuronCores, SPMD):
  - Shard the h axis (128) into 8 slabs of 16 rows; windows (4^3) stay core-local.
  - Host-side prep: per-core channel-major bf16 slab of x (with h halo) for the
    dwc conv; natural fp32 slab for the residual; pre-transposed weights.
  - Phase A (dwc conv): per (d, h-row) matmul  Z[w, o] = sum_c x[c,*] * w27[c, o]
    (o = 27 taps), then 27 free-dim-shifted DVE adds produce dnx in a
    window-friendly layout. w+-1 shifts are pre-materialized by two DMA copies.
  - Phase B (cluster attention): all matmuls on 64-wide fea tiles; softmax over
    windows needs two tiny AllReduces (sum of squares; exp-sums + soft counts).
    No max-subtraction (logits are tiny; validated numerically).
  - Phase C (upc conv + residual): out volume -> DRAM bounce -> ov tile (d on
    partitions) -> 27-row im2col built by DMA -> one matmul per (d,h) row,
    + fp32 residual, streamed back to HBM.  h-halo of ov comes from an
    AllGather of window-face slices.
"""

import os
import sys
from contextlib import ExitStack
from dataclasses import dataclass

import numpy as np

for _p in ("/opt/trn_rl_repo",):
    if os.path.isdir(_p) and _p not in sys.path:
        sys.path.insert(0, _p)

os.environ.setdefault("MYCRO_LOCAL_CACHE", "1")

import ml_dtypes  # noqa: E402
import concourse.bass as bass  # noqa: E402
import concourse.tile as tile  # noqa: E402
from concourse import bacc, mybir  # noqa: E402
from concourse import bass_utils  # noqa: E402
from concourse.masks import make_identity  # noqa: E402

F32 = mybir.dt.float32
BF16 = mybir.dt.bfloat16
AF = mybir.ActivationFunctionType
ALU = mybir.AluOpType

C = 96          # channels
P = 4           # window edge
FEAD = 64       # P^3
K = 64          # clusters
NCORES = 8
W = 128         # w extent == partition count


@dataclass(frozen=True)
class Cfg:
    D: int = 32      # d extent
    HL: int = 16     # h rows per core (total H = 8*HL)

    @property
    def HZ(self):     # h rows incl halo
        return self.HL + 2

    @property
    def DZ(self):     # d extent incl halo
        return self.D + 2

    @property
    def WD(self):
        return self.D // P

    @property
    def WHL(self):
        return self.HL // P

    @property
    def WWN(self):
        return W // P

    @property
    def NLOC(self):  # windows per core
        return self.WD * self.WHL * self.WWN

    @property
    def NT(self):    # 128-row tiles of local windows
        return self.NLOC // 128


FULL = Cfg()

_BUILD_CACHE: dict = {}


def mkap(base, extra_off, dims):
    """Manual AP on the same tensor; offsets/steps in flat element units
    (partition pitch == free size)."""
    return bass.AP(tensor=base.tensor, offset=base.offset + extra_off, ap=dims)


def build_module(cfg: Cfg):
    D, HL, HZ, DZ = cfg.D, cfg.HL, cfg.HZ, cfg.DZ
    WD, WHL, WWN, NLOC, NT = cfg.WD, cfg.WHL, cfg.WWN, cfg.NLOC, cfg.NT
    WZP = W + 2
    PHZ = DZ * HZ           # per-o plane size in Z storage

    nc = bacc.Bacc("TRN2", target_bir_lowering=False, debug=False,
                   num_devices=NCORES)

    # ---------------- I/O ----------------
    xt = nc.dram_tensor("xt", [C, D, HZ, W], BF16, kind="ExternalInput").ap()
    xr = nc.dram_tensor("xr", [W, D, HL, C], BF16, kind="ExternalInput").ap()
    w27 = nc.dram_tensor("w27", [C, 27], BF16, kind="ExternalInput").ap()
    dwcb = nc.dram_tensor("dwcb", [1, 1], F32, kind="ExternalInput").ap()
    centt = nc.dram_tensor("centt", [FEAD, K], F32, kind="ExternalInput").ap()
    qwt = nc.dram_tensor("qwt", [FEAD, FEAD], F32, kind="ExternalInput").ap()
    qb = nc.dram_tensor("qb", [FEAD, 1], F32, kind="ExternalInput").ap()
    kvk = nc.dram_tensor("kvk", [FEAD + 1, FEAD], F32, kind="ExternalInput").ap()
    kvv = nc.dram_tensor("kvv", [FEAD + 1, FEAD], F32, kind="ExternalInput").ap()
    upw = nc.dram_tensor("upw", [27, C], BF16, kind="ExternalInput").ap()
    y = nc.dram_tensor("y", [W, D, HL, C], BF16, kind="ExternalOutput").ap()

    with tile.TileContext(nc) as tc, ExitStack() as ctx:
        _body(ctx, tc, cfg, xt, xr, w27, dwcb, centt, qwt, qb, kvk, kvv, upw, y)
    nc.compile()
    return nc


def _body(ctx, tc, cfg, xt, xr, w27, dwcb, centt, qwt, qb, kvk, kvv, upw, y):
    nc = tc.nc
    D, HL, HZ, DZ = cfg.D, cfg.HL, cfg.HZ, cfg.DZ
    WD, WHL, WWN, NLOC, NT = cfg.WD, cfg.WHL, cfg.WWN, cfg.NLOC, cfg.NT
    WZP = W + 2
    PHZ = DZ * HZ
    EDGE = WWN * WD * 16          # per-edge halo elements (ww, wd, pd, pw)

    consts = ctx.enter_context(tc.tile_pool(name="consts", bufs=1))
    small = ctx.enter_context(tc.tile_pool(name="small", bufs=1))
    dram = ctx.enter_context(tc.tile_pool(name="dram", bufs=1, space="DRAM"))
    psA = ctx.enter_context(tc.tile_pool(name="psA", bufs=4, space="PSUM"))

    # ---------------- constants ----------------
    w27s = consts.tile([C, 27], BF16)
    nc.sync.dma_start(w27s[:], w27[:])
    upws = consts.tile([27, C], BF16)
    nc.sync.dma_start(upws[:], upw[:])
    centts = consts.tile([FEAD, K], F32)
    nc.sync.dma_start(centts[:], centt[:])
    qwts = consts.tile([FEAD, FEAD], F32)
    nc.sync.dma_start(qwts[:], qwt[:])
    qbs = consts.tile([FEAD, 1], F32)
    nc.sync.dma_start(qbs[:], qb[:])
    kvks = consts.tile([FEAD + 1, FEAD], F32)
    nc.sync.dma_start(kvks[:], kvk[:])
    kvvs = consts.tile([FEAD + 1, FEAD], F32)
    nc.sync.dma_start(kvvs[:], kvv[:])
    dwcb1 = consts.tile([1, 1], F32)
    nc.sync.dma_start(dwcb1[:], dwcb[:])
    dwcbb = consts.tile([W, 1], F32)
    nc.gpsimd.partition_broadcast(dwcbb[:], dwcb1[:])
    ident = consts.tile([FEAD, FEAD], F32)
    make_identity(nc, ident[:])

    # persistent mid-size tensors
    dnx = small.tile([W, P, P, WD, WHL], F32)        # (w; pd, ph, wd, wh)
    feat = small.tile([FEAD, NLOC], F32)             # fea^T (j, n) n=(ww,wd,wh)
    fa = small.tile([128, NT, FEAD + 1], F32)        # fea (n, j | 1)
    ee = small.tile([128, NT, K], F32)               # exp(logits) (n, k)
    sq = small.tile([FEAD, NLOC], F32)               # scratch for Square
    ov = small.tile([DZ, HZ, WZP], BF16)             # out volume (d; h, w)
    WQ2 = W // P

    # DRAM bounce buffers
    ar1_in = dram.tile([FEAD, 1], F32)
    ar1_out = dram.tile([FEAD, 1], F32, addr_space="Shared")
    ar2_in = dram.tile([K, FEAD + 1], F32)
    ar2_out = dram.tile([K, FEAD + 1], F32, addr_space="Shared")
    ovd = dram.tile([NLOC, FEAD], BF16)
    ovd2 = dram.tile([WHL, WD, P, P, W], BF16)  # (wh, wd, pd, ph, w) true-w order
    ag_in = dram.tile([2, EDGE], BF16)
    ag_out = dram.tile([NCORES, 2, EDGE], BF16, addr_space="Shared")

    # ================= PHASE A: dwc conv =================
    with tc.tile_pool(name="xin", bufs=3) as xpool, \
         tc.tile_pool(name="zps", bufs=4, space="PSUM") as zps, \
         tc.tile_pool(name="zsb", bufs=1) as zpool:

        zsb = zpool.tile([W, 9, 3, DZ, HZ], F32)     # Z (w; g, ow, dz, hz)
        zp = zpool.tile([W, 9, DZ, HZ], F32)         # Z shifted w+1 (ow=+1 taps)
        zm = zpool.tile([W, 9, DZ, HZ], F32)         # Z shifted w-1 (ow=-1 taps)

        # boundary zeros: d-halo planes of Z; full zero-init of the shifted
        # copies (covers their w-edge and d-halo rows in one go)
        nc.vector.memset(zsb[:, :, :, 0, :], 0.0)
        nc.vector.memset(zsb[:, :, :, DZ - 1, :], 0.0)
        nc.vector.memset(zp[:], 0.0)
        nc.vector.memset(zm[:], 0.0)

        for d in range(D):
            xin = xpool.tile([C, HZ, W], BF16)
            (nc.sync if d % 2 == 0 else nc.scalar).dma_start(
                xin[:], xt[:, d, :, :])
            for hb0 in range(0, HZ, 4):
                nr = min(4, HZ - hb0)
                ps = zps.tile([W, 4 * 27], F32, tag="zps")
                for i in range(nr):
                    nc.tensor.matmul(ps[:, i * 27:(i + 1) * 27],
                                     lhsT=xin[:, hb0 + i, :], rhs=w27s[:],
                                     start=True, stop=True)
                # copy psum -> zsb (o-major planes), strided dest
                src = ps[:, 0:nr * 27].rearrange("p (h g w3) -> p h g w3",
                                                 g=9, w3=3)
                dst = mkap(zsb[:], (d + 1) * HZ + hb0,
                           [[9 * 3 * PHZ, W], [1, nr], [3 * PHZ, 9], [PHZ, 3]])
                if (d * 5 + hb0 // 4) % 2 == 0:
                    nc.scalar.copy(dst, src)
                else:
                    nc.vector.tensor_copy(dst, src)

        # w-shifted copies.  Partitions are in permuted order w' = pw*32+ww
        # (true w = 4*ww+pw), so a +-1 shift in true w becomes two
        # consecutive-partition-range copies (+-32, and the pw wraparound).
        WQ = W // P   # 32
        for half in range(2):
            dz0 = 1 + half * (D // 2)
            ndz = D // 2
            span = [1, ndz * HZ]

            def shcopy(dstt, dst_fsz, dst_p0, nparts, src_p0, owi,
                       _c=[0]):
                _c[0] += 1
                (nc.sync if _c[0] % 2 == 0 else nc.scalar).dma_start(
                    out=mkap(dstt, dst_p0 * 9 * PHZ + dz0 * HZ,
                             [[9 * PHZ, nparts], [PHZ, 9], list(span)]),
                    in_=mkap(zsb[:], src_p0 * 27 * PHZ + owi * PHZ + dz0 * HZ,
                             [[27 * PHZ, nparts], [3 * PHZ, 9], list(span)]))

            # zp[p] = Z[w(p)+1]: dest [0,3*WQ) <- src [WQ,4*WQ);
            #                    dest [3*WQ, 4*WQ-1) <- src [1, WQ)
            shcopy(zp[:], 9 * PHZ, 0, 3 * WQ, WQ, 2)
            shcopy(zp[:], 9 * PHZ, 3 * WQ, WQ - 1, 1, 2)
            # zm[p] = Z[w(p)-1]: dest [WQ,4*WQ) <- src [0,3*WQ);
            #                    dest [1, WQ) <- src [3*WQ, 4*WQ-1)
            shcopy(zm[:], 9 * PHZ, WQ, 3 * WQ, 0, 0)
            shcopy(zm[:], 9 * PHZ, 1, WQ - 1, 3 * WQ, 0)

        # shifted-sum -> dnx, split per pd so every DVE op has <=3 free dims
        def zterm(g, ow, pd):
            od, oh = g // 3 - 1, g % 3 - 1
            if ow == 1:
                base, plane = zsb[:], (g * 3 + 1) * PHZ
            elif ow == 2:
                base, plane = zp[:], g * PHZ
            else:
                base, plane = zm[:], g * PHZ
            off = plane + (pd + od + 1) * HZ + (oh + 1)
            return mkap(base, off,
                        [[base.ap[0][0], W], [1, P], [4 * HZ, WD], [P, WHL]])

        for pd in range(P):
            acc = dnx[:, pd, :, :, :]
            nc.vector.tensor_add(acc, zterm(0, 0, pd), zterm(0, 1, pd))
            for g in range(9):
                for ow in range(3):
                    if g == 0 and ow in (0, 1):
                        continue
                    nc.vector.tensor_add(acc, acc, zterm(g, ow, pd))
        # + dwc bias (flat 2-D view; per-partition scalar broadcast)
        nc.vector.tensor_scalar_add(
            dnx[:].rearrange("p a b c d -> p (a b c d)"),
            dnx[:].rearrange("p a b c d -> p (a b c d)"), dwcbb[:])

    # ================= window partition =================
    # feat[j, n]; n = ww*(WD*WHL) + wd*WHL + wh ; j = pd*16 + ph*4 + pw
    # batched: one DMA per (pd, ph) covering the 4 pw partitions
    NWIN_D = WD * WHL
    fsz = P * P * NWIN_D                # free size of dnx
    for pd in range(P):
        for ph in range(P):
            j0 = pd * 16 + ph * 4
            off = pd * (P * NWIN_D) + ph * NWIN_D
            src = mkap(dnx[:], off,
                       [[WQ2 * fsz, P], [fsz, WWN], [1, NWIN_D]])
            (nc.sync if (pd * P + ph) % 2 == 0 else
             nc.scalar).dma_start(out=feat[j0:j0 + 4, :], in_=src)

    # ---- sumsq -> AllReduce -> rnorm ----
    su = small.tile([FEAD, 1], F32)
    nc.scalar.activation(sq[:], feat[:], AF.Square, accum_out=su[:])
    nc.sync.dma_start(ar1_in[:], su[:])
    nc.gpsimd.collective_compute("AllReduce", ALU.add,
                                 replica_groups=[list(range(NCORES))],
                                 ins=[ar1_in.opt()], outs=[ar1_out.opt()])
    rn = small.tile([FEAD, 1], F32)
    nc.sync.dma_start(rn[:], ar1_out[:])
    nc.scalar.sqrt(rn[:], rn[:])
    nc.vector.tensor_scalar_max(rn[:], rn[:], 1e-12)
    nc.vector.reciprocal(rn[:], rn[:])

    # fea (n-major) tiles + ones column, via PE transpose
    nc.vector.memset(fa[:, :, FEAD:FEAD + 1], 1.0)
    for t in range(NT):
        pt = psA.tile([128, FEAD], F32, tag="pb")
        nc.tensor.transpose(pt[:], feat[:, t * 128:(t + 1) * 128], ident[:])
        nc.scalar.copy(fa[:, t, 0:FEAD], pt[:])

    # ---- logits, exp, A matrix ----
    cst = small.tile([FEAD, K], F32)
    nc.vector.tensor_scalar_mul(cst[:], centts[:], rn[:])
    for t in range(NT):
        lg = psA.tile([128, K], F32, tag="pb")
        nc.tensor.matmul(lg[:], lhsT=feat[:, t * 128:(t + 1) * 128],
                         rhs=cst[:], start=True, stop=True)
        nc.scalar.activation(ee[:, t, :], lg[:], AF.Exp)

    aps = psA.tile([K, FEAD + 1], F32, tag="pb")
    for t in range(NT):
        nc.tensor.matmul(aps[:], lhsT=ee[:, t, :], rhs=fa[:, t, :],
                         start=(t == 0), stop=(t == NT - 1))
    asb = small.tile([K, FEAD + 1], F32)
    nc.scalar.copy(asb[:], aps[:])
    nc.sync.dma_start(ar2_in[:], asb[:])
    nc.gpsimd.collective_compute("AllReduce", ALU.add,
                                 replica_groups=[list(range(NCORES))],
                                 ins=[ar2_in.opt()], outs=[ar2_out.opt()])
    ag = small.tile([K, FEAD + 1], F32)
    nc.sync.dma_start(ag[:], ar2_out[:])

    # ---- new centroids, k/v, q ----
    rs = small.tile([K, 1], F32)
    nc.vector.reciprocal(rs[:], ag[:, FEAD:FEAD + 1])
    nc.vector.tensor_scalar_mul(ag[:, 0:FEAD], ag[:, 0:FEAD], rs[:])
    nct = small.tile([FEAD + 1, K], F32)
    ncp = psA.tile([FEAD, K], F32, tag="pb")
    nc.tensor.transpose(ncp[:], ag[:, 0:FEAD], ident[:])
    nc.vector.tensor_scalar_mul(nct[0:FEAD, :], ncp[:], rn[:])
    nc.vector.memset(nct[FEAD:FEAD + 1, :], 1.0)

    kt = small.tile([FEAD, K], F32)
    kp = psA.tile([FEAD, K], F32, tag="pb")
    nc.tensor.matmul(kp[:], lhsT=kvks[:], rhs=nct[:], start=True, stop=True)
    nc.scalar.copy(kt[:], kp[:])

    va = small.tile([K, FEAD + 1], F32)
    vp = psA.tile([K, FEAD], F32, tag="pb")
    nc.tensor.matmul(vp[:], lhsT=nct[:], rhs=kvvs[:], start=True, stop=True)
    nc.scalar.copy(va[:, 0:FEAD], vp[:])
    nc.vector.memset(va[:, FEAD:FEAD + 1], 1.0)

    qws = small.tile([FEAD, FEAD], F32)
    nc.vector.tensor_scalar_mul(qws[:], qwts[:], rn[:])
    qt = small.tile([FEAD, NLOC], F32)
    e2 = small.tile([K, NLOC], F32)
    CH = 512 if NLOC % 512 == 0 else 128
    for h0 in range(0, NLOC, CH):
        qp = psA.tile([FEAD, CH], F32, tag="pb")
        nc.tensor.matmul(qp[:], lhsT=qws[:], rhs=feat[:, h0:h0 + CH],
                         start=True, stop=True)
        nc.scalar.activation(qt[:, h0:h0 + CH], qp[:], AF.Identity, bias=qbs[:])
    for h0 in range(0, NLOC, CH):
        qk = psA.tile([K, CH], F32, tag="pb")
        nc.tensor.matmul(qk[:], lhsT=kt[:], rhs=qt[:, h0:h0 + CH],
                         start=True, stop=True)
        nc.scalar.activation(e2[:, h0:h0 + CH], qk[:], AF.Exp,
                             scale=float(FEAD) ** -0.5)

    # ---- attention output tiles -> DRAM dump + halo faces ----
    for t in range(NT):
        op = psA.tile([128, FEAD + 1], F32, tag="pb")
        nc.tensor.matmul(op[:], lhsT=e2[:, t * 128:(t + 1) * 128], rhs=va[:],
                         start=True, stop=True)
        rc = small.tile([128, 1], F32, tag="rc")
        nc.vector.reciprocal(rc[:], op[:, FEAD:FEAD + 1])
        ot = small.tile([128, FEAD], BF16, tag="ot")
        nc.vector.tensor_scalar_mul(ot[:], op[:, 0:FEAD], rc[:])
        (nc.sync if t % 2 == 0 else nc.scalar).dma_start(
            ovd[t * 128:(t + 1) * 128, :], ot[:])

    # reorder dump (n, j) -> volume-row layout (wh, wd, pd, ph, w) in DRAM
    for wh in range(WHL):
        for pd in range(P):
            for ph in range(P):
                ((nc.sync if (pd * P + ph) % 2 == 0 else
                  nc.scalar)).dma_start(
                    out=mkap(ovd2[:],
                             ((wh * WD) * 16 + pd * P + ph) * W,
                             [[16 * W, WD], [P, WWN], [1, P]]),
                    in_=mkap(ovd[:],
                             wh * FEAD + pd * 16 + ph * P,
                             [[WHL * FEAD, WD], [NWIN_D * FEAD, WWN], [1, P]]))
    # halo faces (one contiguous-w call per edge); layout [edge][wd][pd][w]
    for edge in range(2):
        wh_e = 0 if edge == 0 else WHL - 1
        ph_e = 0 if edge == 0 else P - 1
        (nc.sync if edge == 0 else nc.scalar).dma_start(
            out=mkap(ag_in[:], edge * EDGE, [[2 * EDGE, 1], [1, EDGE]]),
            in_=mkap(ovd2[:], ((wh_e * WD) * 16 + ph_e) * W,
                     [[16 * W, WD], [P * W, P], [1, W]]))

    nc.gpsimd.collective_compute("AllGather", ALU.bypass,
                                 replica_groups=[list(range(NCORES))],
                                 ins=[ag_in.opt()], outs=[ag_out.opt()])

    # ---- build ov (bf16): one DMA per (pd, ph) covering all (wd, wh, w) ----
    nc.vector.memset(ov[:], 0.0)
    OVF = HZ * WZP
    for pd in range(P):
        for ph in range(P):
            dst = mkap(ov[:], (pd + 1) * OVF + (ph + 1) * WZP + 1,
                       [[4 * OVF, WD], [P * WZP, WHL], [1, W]])
            src = mkap(ovd2[:], (pd * P + ph) * W,
                       [[16 * W, WD], [WD * 16 * W, WHL], [1, W]])
            (nc.sync if (pd * P + ph) % 2 == 0 else
             nc.scalar).dma_start(out=dst, in_=src)
    # halo rows from AllGather (dynamic rank offsets, edge cores skip);
    # one batched DMA per edge over (wd, pd, w)
    pid = nc.partition_id()
    dstl = mkap(ov[:], 1 * OVF + 0 * WZP + 1,
                [[4 * OVF, WD], [OVF, P], [1, W]])
    srcl = mkap(ag_out[:], (pid - 1) * (2 * EDGE) + 1 * EDGE,
                [[P * W, WD], [W, P], [1, W]])
    nc.gpsimd.dma_start(out=dstl, in_=srcl, cond=(pid >= 1))
    dsth = mkap(ov[:], 1 * OVF + (HZ - 1) * WZP + 1,
                [[4 * OVF, WD], [OVF, P], [1, W]])
    srch = mkap(ag_out[:], (pid + 1) * (2 * EDGE) + 0 * EDGE,
                [[P * W, WD], [W, P], [1, W]])
    nc.gpsimd.dma_start(out=dsth, in_=srch, cond=(pid <= NCORES - 2))

    # ================= PHASE C: upc conv + residual =================
    # im2col built ONCE for the whole volume (27 DMAs); per-d batched
    # bf16 residual loads and y stores (32 + 32 DMAs).
    with tc.tile_pool(name="i2c", bufs=1) as cpool, \
         tc.tile_pool(name="xrp", bufs=4) as xrp, \
         tc.tile_pool(name="yp", bufs=4) as yp, \
         tc.tile_pool(name="psC", bufs=4, space="PSUM") as psC:
        i2c = cpool.tile([27, D * HL * W], BF16)
        for o in range(27):
            od, oh, ow = o // 9 - 1, (o // 3) % 3 - 1, o % 3 - 1
            src = mkap(ov[:], (od + 1) * OVF + (oh + 1) * WZP + (ow + 1),
                       [[OVF, D], [WZP, HL], [1, W]])
            eng = nc.sync if o % 2 == 0 else nc.scalar
            eng.dma_start(out=i2c[o:o + 1, :], in_=src)
        for d in range(D):
            xrd = xrp.tile([W, HL, C], BF16)
            eng = nc.sync if d % 2 == 0 else nc.scalar
            eng.dma_start(
                out=xrd[:],
                in_=mkap(xr[:], d * HL * C,
                         [[D * HL * C, W], [1, HL * C]]))
            ysd = yp.tile([W, HL, C], BF16)
            for hq in range(HL // 4):
                psc = psC.tile([W, 4 * C], F32, tag="psc")
                for i in range(4):
                    h = hq * 4 + i
                    nc.tensor.matmul(
                        psc[:, i * C:(i + 1) * C],
                        lhsT=i2c[:, (d * HL + h) * W:(d * HL + h + 1) * W],
                        rhs=upws[:], start=True, stop=True)
                nc.vector.tensor_add(
                    ysd[:, hq * 4:(hq + 1) * 4, :],
                    psc[:].rearrange("p (a b) -> p a b", a=4),
                    xrd[:, hq * 4:(hq + 1) * 4, :])
            eng2 = nc.scalar if d % 2 == 0 else nc.sync
            eng2.dma_start(
                out=mkap(y[:], d * HL * C,
                         [[D * HL * C, W], [1, HL * C]]),
                in_=ysd[:])


# ======================= host side =======================

def _prep_inputs(cfg: Cfg, inputs):
    x = np.asarray(inputs["x"], np.float32)[0]          # (D, H, W, C)
    D, H = cfg.D, NCORES * cfg.HL
    assert x.shape == (D, H, W, C), x.shape
    HLp = cfg.HL

    xpad = np.zeros((D, H + 2, W, C), np.float32)
    xpad[:, 1:H + 1] = x
    dwc_w = np.asarray(inputs["dwc_w"], np.float32)
    upc_w = np.asarray(inputs["upc_w"], np.float32)
    q_w = np.asarray(inputs["q_w"], np.float32)
    kv_w = np.asarray(inputs["kv_w"], np.float32)
    q_b = np.asarray(inputs["q_b"], np.float32)
    kv_b = np.asarray(inputs["kv_b"], np.float32)
    upc_b = np.asarray(inputs["upc_b"], np.float32)
    cent = np.asarray(inputs["centroids"], np.float32)

    w27 = np.ascontiguousarray(dwc_w[0].reshape(C, 27)).astype(ml_dtypes.bfloat16)
    upwt = np.ascontiguousarray(upc_w[:, 0].reshape(C, 27).T).astype(ml_dtypes.bfloat16)
    dwcb = np.asarray(inputs["dwc_b"], np.float32).reshape(1, 1)
    centT = np.ascontiguousarray(cent.T)
    qwT = np.ascontiguousarray(q_w.T)
    qbv = q_b.reshape(FEAD, 1)
    kvk = np.concatenate([kv_w[0:FEAD].T, kv_b[None, 0:FEAD]], 0)
    kvv = np.concatenate([kv_w[FEAD:2 * FEAD].T, kv_b[None, FEAD:2 * FEAD]], 0)
    kvk = np.ascontiguousarray(kvk)
    kvv = np.ascontiguousarray(kvv)

    in_maps = []
    for k in range(NCORES):
        h0 = k * HLp
        slab = xpad[:, h0:h0 + HLp + 2]                  # (D, HZ, W, C)
        worder = np.array([4 * (p % 32) + p // 32 for p in range(W)])
        slab = slab[:, :, worder, :]
        xt = np.ascontiguousarray(slab.transpose(3, 0, 1, 2)).astype(
            ml_dtypes.bfloat16)
        xrs = x[:, h0:h0 + HLp] + upc_b[None, None, None, :]
        xrs = np.ascontiguousarray(xrs.transpose(2, 0, 1, 3))  # (W, D, HL, C)
        in_maps.append({
            "xt": xt, "xr": xrs.astype(ml_dtypes.bfloat16), "w27": w27,
            "dwcb": dwcb,
            "centt": centT, "qwt": qwT, "qb": qbv, "kvk": kvk, "kvv": kvv,
            "upw": upwt,
        })
    return in_maps


def _get_built(cfg: Cfg):
    if cfg not in _BUILD_CACHE:
        _BUILD_CACHE[cfg] = build_module(cfg)
    return _BUILD_CACHE[cfg]


def _postprocess(cfg: Cfg, res):
    # per-core y is (W, D, HL, C) bf16 -> (D, HL, W, C) f32, concat on h
    ys = [np.asarray(res.results[k]["y"]).astype(np.float32)
          .transpose(1, 2, 0, 3) for k in range(NCORES)]
    yfull = np.concatenate(ys, axis=1)                   # (D, H, W, C)
    return yfull[None]


def kernel(**inputs):
    cfg = FULL
    nc = _get_built(cfg)
    in_maps = _prep_inputs(cfg, inputs)
    res = bass_utils.run_bass_kernel_spmd(nc, in_maps,
                                          core_ids=list(range(NCORES)))
    return _postprocess(cfg, res)

